# revision 1
# baseline (speedup 1.0000x reference)
"""Linear attention (ELU+1 feature map) on 8 TRN2 NeuronCores.

Reference math (per batch b):
    q,k,v = x @ W{q,k,v}.T + b;   q,k -> elu(.)+1
    kv[h,d,e] = sum_t k[t,h,d] v[t,h,e];   ks[h,d] = sum_t k[t,h,d]
    out = ((q kv) / clip(q . ks, 1e-6)) @ Wo.T + bo

Sharding: the 16384 tokens are split into 8 contiguous chunks of 2048; core c
owns batch c//2, T-half c%2. kv/ks are partial sums over the core's tokens,
AllReduce-summed within core pairs {0,1},{2,3},{4,5},{6,7} (one pair = one
batch, adjacent NeuronCores). Everything else is embarrassingly parallel, so
the only communication is a 520 KB pair AllReduce that overlaps the q
projection.

Per-core device program (S=2048 local tokens; a "pair" p = 2 heads = 128
channels; all layouts chosen so no on-device transposes are ever needed):
  phase 1: k,v projections in token-major layout via matmul(lhsT=xT block,
           rhs=W.T stripe). v is written into an interleaved pair layout with
           two ones-columns per pair (single strided 3D-AP copy per PSUM
           chunk), so ONE N=130 matmul per (pair, token-tile) produces both
           the kv outer-product block and the k-sum column, feature-major.
           Partial kv products are DVE-accumulated into SBUF (PSUM
           accumulation groups interleaved within a bank lose their first
           contribution on HW - a sibling group's start=True clears the
           bank's has_written bits).
  phase 2: qT feature-major via matmul(lhsT=Wq.T columns, rhs=xT);
           denominators via a block-diagonal ksum matmul, batched along the
           free dim so one DVE reciprocal serves all pairs; 1/denom is
           broadcast across partitions with a tiny [2,128] selector matmul;
           att = num * (1/denom) stays feature-major and feeds the output
           projection as its stationary operand; y lands token-major in PSUM
           and is copied out through SBUF.

COMPUTE selects the TensorEngine dtype: "f32r" (default) stores f32 bits and
runs the PE in round-trip fp32 mode (full rate at N>=256; ~3.5e-4 rel err),
"bf16" halves the DMA/SBUF footprint (~5.6e-3 rel err), "f32" is the exact
but 4x-slower fallback. Inputs are pre-transposed/sharded on the host; biases
are folded in via an extra ones-row contraction tile only when nonzero (the
bo bias is applied on the host).
"""

import sys
import numpy as np

for _p in ("/opt/trn_rl_repo", "/opt/pypackages"):
    if _p not in sys.path:
        sys.path.append(_p)

import concourse.bacc as bacc
import concourse.mybir as mybir
import concourse.tile as tile
from concourse import bass_utils

F32 = mybir.dt.float32
ACTF = mybir.ActivationFunctionType

N_CORES = 8
B, T, C = 4, 4096, 1024
H, D = 16, 64
S = B * T // N_CORES          # 2048 tokens per core
NP = 8                        # head pairs (128 channels each)
TT = S // 128                 # 16 token tiles per core
HALF = S // 2                 # phase-2 half size (1024)
PSTR = 130                    # kv_aug per-pair stride: 128 kv cols + ksum col
                              # + pad col (f32r matmul needs even N)

COMPUTE = "f32r"              # "f32r" | "bf16" | "f32"
DEBUG_DUMPS = False
REPEAT = 1                    # timing only: emit the body N times in one NEFF

_cache = {}


def _cdt():
    """Storage dtype of matmul-feeding tensors (f32r is f32 bits; the PE
    runs it at full rate when N>=256 and the verifier requires producers
    to declare the f32r dtype end-to-end)."""
    return {"bf16": mybir.dt.bfloat16,
            "f32r": mybir.dt.float32r,
            "f32": F32}[COMPUTE]


def _mm(ap):
    return ap


def _msview(ap):
    """Memset target view: walrus rejects Memset on f32r APs, so write the
    same bits through an f32 view."""
    return ap.bitcast(F32) if COMPUTE == "f32r" else ap


def _emit(nc, tc, KT, xt_d, wk_d, wv_d, wq_d, wo_d, cs_d, out_d, dbg=None):
    dbg = dbg or {}
    cdt = _cdt()
    res_xt = COMPUTE == "bf16"   # 2-byte xT fits SBUF for both phases
    span = HALF if COMPUTE == "bf16" else S // 4
    nchk = span // 512
    Relu, Exp = ACTF.Relu, ACTF.Exp
    WB = 2 * KT  # weight slots: wk+wv in phase 1, reused by wq+wo in phase 2

    with (
        tc.tile_pool(name="wpool", bufs=1) as wpool,
        tc.tile_pool(name="persist", bufs=1) as sb,
        tc.tile_pool(name="dram", bufs=1, space="DRAM") as dram,
    ):
        wk_sb = []
        wv_sb = []
        for ct in range(KT):
            w = wpool.tile([128, C], cdt, tag="w", bufs=WB, name=f"wk{ct}")
            nc.gpsimd.dma_start(w[:], wk_d[ct * 128:(ct + 1) * 128, :])
            wk_sb.append(w)
        for ct in range(KT):
            w = wpool.tile([128, C], cdt, tag="w", bufs=WB, name=f"wv{ct}")
            nc.gpsimd.dma_start(w[:], wv_d[ct * 128:(ct + 1) * 128, :])
            wv_sb.append(w)

        csel = sb.tile([2, 128], cdt, tag="csel", name="csel")
        nc.sync.dma_start(csel[:], cs_d[:])

        kvagg = sb.tile([128, NP * PSTR], F32, tag="kvagg", name="kvagg")

        # ------------- phase 1: k/v projections + kv aggregation -------------
        # NOTE: PSUM accumulation groups interleaved within one bank are
        # broken on HW (a sibling group's start=True clears the bank's
        # has_written bits), so kv partial products are single-shot matmuls
        # accumulated into SBUF by the DVE instead.
        with (
            tc.tile_pool(name="p1sb", bufs=1) as p1,
            tc.tile_pool(name="p1ps", bufs=1, space="PSUM") as ps1,
        ):
            nc.gpsimd.memset(kvagg[:], 0.0)

            # xT stripes: one efficient full-row DMA each (the per-token-tile
            # [128,128] block loads were 512 B/line descriptor-dominated).
            # bf16: allocated from the persistent pool and reused in phase 2.
            xs_pool = sb if res_xt else p1
            xs_sb = []
            for ct in range(KT):
                xst = xs_pool.tile([128, S], cdt, tag="xs", bufs=KT,
                                   name=f"xs{ct}")
                nc.gpsimd.dma_start(xst[:], xt_d[ct * 128:(ct + 1) * 128, :])
                xs_sb.append(xst)

            for tt in range(TT):
                t0 = tt * 128
                xb = [xs_sb[ct][:, t0:t0 + 128] for ct in range(KT)]

                ktok = p1.tile([128, C], cdt, tag="ktok", bufs=3,
                               name=f"ktok{tt}")
                kps, t1s, t2s = [], [], []
                for ch in range(2):
                    kp = ps1.tile([128, 512], F32, tag="ps", bufs=4,
                                  name=f"kp{tt}_{ch}")
                    for ct in range(KT):
                        nc.tensor.matmul(
                            kp[:], _mm(xb[ct]),
                            _mm(wk_sb[ct][:, ch * 512:(ch + 1) * 512]),
                            start=(ct == 0), stop=(ct == KT - 1))
                    kps.append(kp)
                    t1s.append(p1.tile([128, 512], F32, tag="t1", bufs=3,
                                       name=f"t1_{tt}_{ch}"))
                    t2s.append(p1.tile([128, 512], F32, tag="t2", bufs=3,
                                       name=f"t2_{tt}_{ch}"))
                # group by ACT function to avoid per-op table swaps
                for ch in range(2):
                    ks = ktok[:, ch * 512:(ch + 1) * 512]
                    nc.scalar.activation(ks, kps[ch][:], Relu)
                    nc.scalar.activation(t1s[ch][:], kps[ch][:], Relu,
                                         scale=-1.0)
                for ch in range(2):
                    nc.scalar.activation(t2s[ch][:], t1s[ch][:], Exp,
                                         scale=-1.0)
                for ch in range(2):
                    ks = ktok[:, ch * 512:(ch + 1) * 512]
                    nc.vector.tensor_add(ks, ks, t2s[ch][:])

                # v in interleaved pair layout [.. 128 v cols | 2 ones ..]
                # so one N=130 matmul per pair yields kv plus the k-sum.
                # Ones come from a whole-tile memset; v lands via ONE strided
                # 3D-AP copy per psum chunk.
                vaug = p1.tile([128, NP * PSTR], cdt, tag="vaug", bufs=3,
                               name=f"vaug{tt}")
                nc.gpsimd.memset(_msview(vaug[:]), 1.0)
                vau3 = vaug.rearrange("p (g c) -> p g c", c=PSTR)
                for ch in range(2):
                    vp = ps1.tile([128, 512], F32, tag="ps", bufs=4,
                                  name=f"vp{tt}_{ch}")
                    for ct in range(KT):
                        nc.tensor.matmul(
                            vp[:], _mm(xb[ct]),
                            _mm(wv_sb[ct][:, ch * 512:(ch + 1) * 512]),
                            start=(ct == 0), stop=(ct == KT - 1))
                    nc.vector.tensor_copy(
                        vau3[:, ch * 4:(ch + 1) * 4, 0:128],
                        vp[:].rearrange("p (g c) -> p g c", c=128))

                for g in range(3):
                    p0, p1n = 3 * g, min(3 * g + 3, NP)
                    kvt = ps1.tile([128, (p1n - p0) * PSTR], F32, tag="kvt",
                                   bufs=3, name=f"kvt{tt}_{g}",
                                   padded_shape=[128, 3 * PSTR])
                    for p in range(p0, p1n):
                        j = p - p0
                        nc.tensor.matmul(
                            kvt[:, j * PSTR:(j + 1) * PSTR],
                            _mm(ktok[:, p * 128:(p + 1) * 128]),
                            _mm(vaug[:, p * PSTR:(p + 1) * PSTR]),
                            start=True, stop=True)
                    nc.vector.tensor_add(
                        kvagg[:, p0 * PSTR:p1n * PSTR],
                        kvagg[:, p0 * PSTR:p1n * PSTR], kvt[:])

                if tt == 0 and "ktok0" in dbg:
                    kd = p1.tile([128, C], F32, tag="ktd", name="ktd")
                    nc.vector.tensor_copy(kd[:], ktok[:])
                    nc.sync.dma_start(dbg["ktok0"][:], kd[:])
                    vd = p1.tile([128, C], F32, tag="vtd", name="vtd")
                    nc.vector.tensor_copy(vd[:], vtok[:])
                    nc.sync.dma_start(dbg["vtok0"][:], vd[:])


        # ------------- pair AllReduce ----------------------------------------
        bounce_in = dram.tile([128, NP * PSTR], F32, name="bounce_in")
        bounce_out = dram.tile([128, NP * PSTR], F32, name="bounce_out")
        nc.sync.dma_start(bounce_in[:], kvagg[:])
        nc.gpsimd.collective_compute(
            "AllReduce", mybir.AluOpType.add,
            ins=[bounce_in.opt()], outs=[bounce_out.opt()],
            replica_groups=[[2 * i, 2 * i + 1] for i in range(N_CORES // 2)])
        kvcoll = sb.tile([128, NP * PSTR], F32, tag="kvcoll", name="kvcoll")
        nc.sync.dma_start(kvcoll[:], bounce_out[:])
        if "kvcoll" in dbg:
            nc.sync.dma_start(dbg["kvcoll"][:], kvcoll[:])
            nc.sync.dma_start(dbg["kvagg"][:], kvagg[:])

        # phase-2 weights (reuse the phase-1 weight slots)
        wq_sb = []
        wo_sb = []
        for ct in range(KT):
            w = wpool.tile([128, C], cdt, tag="w", bufs=WB, name=f"wq{ct}")
            nc.gpsimd.dma_start(w[:], wq_d[ct * 128:(ct + 1) * 128, :])
            wq_sb.append(w)
        for ct in range(NP):
            w = wpool.tile([128, C], cdt, tag="w", bufs=WB, name=f"wo{ct}")
            nc.gpsimd.dma_start(w[:], wo_d[ct * 128:(ct + 1) * 128, :])
            wo_sb.append(w)

        # block-diagonal kv (cross-head junk zeroed) + ksum column tiles
        kvblk = []
        ksb = []
        for p in range(NP):
            c0 = p * PSTR
            kb = sb.tile([128, 128], cdt, tag="kvblk", bufs=NP,
                         name=f"kvblk{p}")
            nc.gpsimd.memset(_msview(kb[:]), 0.0)
            nc.vector.tensor_copy(kb[0:64, 0:64], kvcoll[0:64, c0:c0 + 64])
            nc.vector.tensor_copy(kb[64:128, 64:128],
                                  kvcoll[64:128, c0 + 64:c0 + 128])
            kvblk.append(kb)
            kt = sb.tile([128, 2], cdt, tag="ksb", bufs=NP, name=f"ksb{p}")
            nc.gpsimd.memset(_msview(kt[:]), 0.0)
            nc.vector.tensor_copy(kt[0:64, 0:1],
                                  kvcoll[0:64, c0 + 128:c0 + 129])
            nc.vector.tensor_copy(kt[64:128, 1:2],
                                  kvcoll[64:128, c0 + 128:c0 + 129])
            ksb.append(kt)

        # ------------- phase 2: q, attention, output projection --------------
        with (
            tc.tile_pool(name="p2sb", bufs=1) as p2,
            tc.tile_pool(name="p2ps", bufs=1, space="PSUM") as ps2,
        ):
            for hv in range(S // span):
                hb = hv * span
                if res_xt:
                    xh = [xs_sb[ct][:, hb:hb + span] for ct in range(KT)]
                else:
                    xh = []
                    for ct in range(KT):
                        xht = p2.tile([128, span], cdt, tag="xh",
                                      bufs=KT + 1, name=f"xh{hv}_{ct}")
                        nc.sync.dma_start(
                            xht[:],
                            xt_d[ct * 128:(ct + 1) * 128, hb:hb + span])
                        xh.append(xht)

                dnb = p2.tile([2, NP * span], F32, tag="dnb", bufs=1,
                              name=f"dnb{hv}")
                qts = []
                for p in range(NP):
                    qt = p2.tile([128, span], cdt, tag="qt", bufs=NP + 1,
                                 name=f"qt{hv}_{p}")
                    qts.append(qt)
                    qps, t1s, t2s = [], [], []
                    for chk in range(nchk):
                        qp = ps2.tile([128, 512], F32, tag="ps", bufs=6,
                                      name=f"qp{hv}_{p}_{chk}")
                        for ct in range(KT):
                            nc.tensor.matmul(
                                qp[:],
                                _mm(wq_sb[ct][:, p * 128:(p + 1) * 128]),
                                _mm(xh[ct][:, chk * 512:(chk + 1) * 512]
                                    if not res_xt else
                                    xs_sb[ct][:, hb + chk * 512:
                                              hb + (chk + 1) * 512]),
                                start=(ct == 0), stop=(ct == KT - 1))
                        qps.append(qp)
                        t1s.append(p2.tile([128, 512], F32, tag="qt1",
                                           bufs=3, name=f"qt1_{hv}_{p}_{chk}"))
                        t2s.append(p2.tile([128, 512], F32, tag="qt2",
                                           bufs=3, name=f"qt2_{hv}_{p}_{chk}"))
                    for chk in range(nchk):
                        qs = qt[:, chk * 512:(chk + 1) * 512]
                        nc.scalar.activation(qs, qps[chk][:], Relu)
                        nc.scalar.activation(t1s[chk][:], qps[chk][:], Relu,
                                             scale=-1.0)
                    for chk in range(nchk):
                        nc.scalar.activation(t2s[chk][:], t1s[chk][:], Exp,
                                             scale=-1.0)
                    for chk in range(nchk):
                        qs = qt[:, chk * 512:(chk + 1) * 512]
                        nc.vector.tensor_add(qs, qs, t2s[chk][:])

                    for chk in range(nchk):
                        dn = ps2.tile([2, 512], F32, tag="dn", bufs=2,
                                      name=f"dn{hv}_{p}_{chk}")
                        nc.tensor.matmul(
                            dn[:], _mm(ksb[p][:]),
                            _mm(qt[:, chk * 512:(chk + 1) * 512]),
                            start=True, stop=True)
                        nc.vector.tensor_scalar_max(
                            dnb[:, p * span + chk * 512:
                                p * span + (chk + 1) * 512], dn[:], 1e-6)

                recb = p2.tile([2, NP * span], cdt, tag="recb", bufs=1,
                               name=f"recb{hv}")
                with nc.allow_low_precision(reason="recip of clipped denom"):
                    nc.vector.reciprocal(recb[:], dnb[:])

                att = []
                for p in range(NP):
                    qt = qts[p]
                    at = p2.tile([128, span], cdt, tag="att", bufs=NP + 1,
                                 name=f"att{hv}_{p}")
                    for chk in range(nchk):
                        nm = ps2.tile([128, 512], F32, tag="ps", bufs=6,
                                      name=f"nm{hv}_{p}_{chk}")
                        nc.tensor.matmul(
                            nm[:], _mm(kvblk[p][:]),
                            _mm(qt[:, chk * 512:(chk + 1) * 512]),
                            start=True, stop=True)
                        rp = ps2.tile([128, 512], F32, tag="ps", bufs=6,
                                      name=f"rp{hv}_{p}_{chk}")
                        nc.tensor.matmul(
                            rp[:], _mm(csel[:]),
                            _mm(recb[:, p * span + chk * 512:
                                     p * span + (chk + 1) * 512]),
                            start=True, stop=True)
                        ats = at[:, chk * 512:(chk + 1) * 512]
                        nc.scalar.copy(ats, nm[:])
                        nc.vector.tensor_mul(ats, ats, rp[:])
                    att.append(at)

                for mt in range(span // 128):
                    r0 = hb + mt * 128
                    for ch in range(2):
                        yp = ps2.tile([128, 512], F32, tag="ps", bufs=6,
                                      name=f"yp{hv}_{mt}_{ch}")
                        for p in range(NP):
                            nc.tensor.matmul(
                                yp[:],
                                _mm(att[p][:, mt * 128:(mt + 1) * 128]),
                                _mm(wo_sb[p][:, ch * 512:(ch + 1) * 512]),
                                start=(p == 0), stop=(p == NP - 1))
                        ysb = p2.tile([128, 512], F32, tag="ysb", bufs=3,
                                      name=f"ysb{hv}_{mt}_{ch}")
                        nc.scalar.copy(ysb[:], yp[:])
                        nc.sync.dma_start(
                            out_d[r0:r0 + 128, ch * 512:(ch + 1) * 512],
                            ysb[:])


def _build(has_bias: bool):
    KT = 9 if has_bias else 8
    KC = KT * 128
    cdt = _cdt()

    nc = bacc.Bacc("TRN2", target_bir_lowering=False, debug=False,
                   num_devices=N_CORES)
    xt_d = nc.dram_tensor("xt", [KC, S], cdt, kind="ExternalInput").ap()
    wk_d = nc.dram_tensor("wkt", [KC, C], cdt, kind="ExternalInput").ap()
    wv_d = nc.dram_tensor("wvt", [KC, C], cdt, kind="ExternalInput").ap()
    wq_d = nc.dram_tensor("wqt", [KC, C], cdt, kind="ExternalInput").ap()
    wo_d = nc.dram_tensor("wot", [KC, C], cdt, kind="ExternalInput").ap()
    cs_d = nc.dram_tensor("csel", [2, 128], cdt, kind="ExternalInput").ap()
    out_d = nc.dram_tensor("out", [S, C], F32, kind="ExternalOutput").ap()
    dbg = {}
    if DEBUG_DUMPS:
        dbg["kvcoll"] = nc.dram_tensor(
            "d_kvcoll", [128, NP * PSTR], F32, kind="ExternalOutput").ap()
        dbg["kvagg"] = nc.dram_tensor(
            "d_kvagg", [128, NP * PSTR], F32, kind="ExternalOutput").ap()
        dbg["ktok0"] = nc.dram_tensor(
            "d_ktok0", [128, C], F32, kind="ExternalOutput").ap()
        dbg["vtok0"] = nc.dram_tensor(
            "d_vtok0", [128, C], F32, kind="ExternalOutput").ap()

    with tile.TileContext(nc) as tc:
        for _ in range(REPEAT):
            _emit(nc, tc, KT, xt_d, wk_d, wv_d, wq_d, wo_d, cs_d, out_d, dbg)
    nc.compile()
    return nc


def _prep_host(inputs, KT):
    """Host-side shard + transpose prep. Returns in_maps for the 8 cores."""
    KC = KT * 128
    npdt = mybir.dt.np(_cdt())
    x = np.asarray(inputs["x"], np.float32).reshape(B * T, C)

    def padw(w, b):
        wt = np.ascontiguousarray(np.asarray(w, np.float32).T)  # [C_in, C_out]
        if KC == C:
            return wt.astype(npdt)
        out = np.zeros((KC, C), np.float32)
        out[:C] = wt
        out[C] = np.asarray(b, np.float32)
        return out.astype(npdt)

    wkt = padw(inputs["Wk"], inputs["bk"])
    wvt = padw(inputs["Wv"], inputs["bv"])
    wqt = padw(inputs["Wq"], inputs["bq"])
    wot = padw(inputs["Wo"], np.zeros(C))   # bo applied on host

    csel = np.zeros((2, 128), np.float32)
    csel[0, :64] = 1.0
    csel[1, 64:] = 1.0
    csel = csel.astype(npdt)

    in_maps = []
    for c in range(N_CORES):
        sh = x[c * S:(c + 1) * S]
        xt = np.zeros((KC, S), np.float32)
        xt[:C] = sh.T
        if KC > C:
            xt[C] = 1.0
        in_maps.append({
            "xt": np.ascontiguousarray(xt.astype(npdt)),
            "wkt": wkt, "wvt": wvt, "wqt": wqt, "wot": wot,
            "csel": csel,
        })
    return in_maps


def _get_nc(has_bias):
    key = (COMPUTE, has_bias, DEBUG_DUMPS, REPEAT)
    if key not in _cache:
        _cache[key] = _build(has_bias)
    return _cache[key]


def kernel(**inputs):
    assert np.asarray(inputs["x"]).shape == (B, T, C)
    has_bias = any(
        np.any(np.asarray(inputs[k])) for k in ("bq", "bk", "bv"))
    nc = _get_nc(has_bias)
    in_maps = _prep_host(inputs, 9 if has_bias else 8)
    res = bass_utils.run_bass_kernel_spmd(
        nc, in_maps, core_ids=list(range(N_CORES)))
    y = np.concatenate(
        [res.results[c]["out"] for c in range(N_CORES)], axis=0)
    y = y.reshape(B, T, C).astype(np.float32)
    bo = np.asarray(inputs["bo"], np.float32)
    if np.any(bo):
        y = y + bo
    return y



# revision 2
# speedup vs baseline: 1.0376x; 1.0376x over previous
"""Linear attention (ELU+1 feature map) on 8 TRN2 NeuronCores — v5.

Algorithm (see v2): bf16 projections; ELU+1 split DVE/ACT/DVE; per-pair
kvT = vtok^T @ ktok and ksum = ktok^T @ ones2 single-shot into PSUM,
DVE-accumulated in f32 (PSUM accumulation groups interleaved within a bank
are broken on HW); pair AllReduce of kv/ksum overlapped with the q
projection; G_p = kv_p @ Wo_p^T folds kv into the output projection;
KS_p (half-masked free-broadcast ksum) gives the denominator already
broadcast to 128 partitions in ONE matmul; qs = qhat * (1/dnB) in place
(DVE reciprocal + Pool multiply); out = qs^T @ G accumulated over pairs.

Scheduling (v5):
  - ONE PSUM pool for the whole kernel. Opening a new pool makes every new
    tile wait on ALL users of the closed pool (release-boundary barrier,
    measured 5.8us at each phase edge); instead phase 1.5's qp reuses the
    kp/vp tag, G/dnB reuse the kvt tag, yp reuses the pp tag, so cross-
    phase waits are per-slot and land exactly on the pipelined tail.
  - ONE transient SBUF pool for the same reason.
  - kv/ksum matmuls of token-tile tt-1 are emitted after the projections
    of tt; kvt has 4 PSUM bufs so the g2 matmul never waits on the DVE
    aggregation adds.
  - input DMAs ride HWDGE queues (sync for wk, scalar for the rest) in
    consumption order -- gpsimd SWDGE DMAs would occupy the Pool engine.
  - phase-2 unpack runs on Pool during phase 1.5; chunk c+1's
    denominator chains interleave with chunk c's output projection.
"""

import sys
import numpy as np

for _p in ("/opt/trn_rl_repo", "/opt/pypackages"):
    if _p not in sys.path:
        sys.path.append(_p)

import concourse.bacc as bacc
import concourse.mybir as mybir
import concourse.tile as tile
from concourse import bass_utils

F32 = mybir.dt.float32
BF16 = mybir.dt.bfloat16
ACTF = mybir.ActivationFunctionType
Alu = mybir.AluOpType

N_CORES = 8
B, T, C = 4, 4096, 1024
H, D = 16, 64
S = B * T // N_CORES          # 2048 tokens per core
NP = 8                        # head pairs (128 channels each)
TT = S // 128                 # 16 token tiles per core
PSTR = 130                    # kv slot: 128 kvT cols + 2 ksum cols
XCH = 512                     # xs DMA token chunk

_cache = {}


def _emit(nc, tc, KT, xt_d, wk_d, wv_d, wq_d, wo_d, out_d):
    Exp = ACTF.Exp

    with (
        tc.tile_pool(name="wkv", bufs=1) as wkv,
        tc.tile_pool(name="wqo", bufs=1) as wqo,
        tc.tile_pool(name="persist", bufs=1) as sb,
        tc.tile_pool(name="trans", bufs=1) as tr,
        tc.tile_pool(name="psum", bufs=1, space="PSUM") as ps,
        tc.tile_pool(name="dram", bufs=1, space="DRAM") as dram,
    ):
        # ---- input DMAs, in consumption order --------------------------
        # sync: wk (gates the very first matmuls); scalar: everything else
        wk_sb, wv_sb = [], []
        for ct in range(KT):
            w = wkv.tile([128, C], BF16, tag="wkv", bufs=2 * KT,
                         name=f"wk{ct}")
            nc.sync.dma_start(w[:], wk_d[ct * 128:(ct + 1) * 128, :])
            wk_sb.append(w)

        xs_sb = [sb.tile([128, S], BF16, tag="xs", bufs=KT, name=f"xs{ct}")
                 for ct in range(KT)]
        for ct in range(KT):
            nc.scalar.dma_start(xs_sb[ct][:, 0:XCH],
                                xt_d[ct * 128:(ct + 1) * 128, 0:XCH])
        for ct in range(KT):
            w = wkv.tile([128, C], BF16, tag="wkv", bufs=2 * KT,
                         name=f"wv{ct}")
            nc.sync.dma_start(w[:], wv_d[ct * 128:(ct + 1) * 128, :])
            wv_sb.append(w)
        for xc in range(1, S // XCH):
            for ct in range(KT):
                nc.sync.dma_start(
                    xs_sb[ct][:, xc * XCH:(xc + 1) * XCH],
                    xt_d[ct * 128:(ct + 1) * 128, xc * XCH:(xc + 1) * XCH])
        wq_sb, wo_sb = [], []
        for ct in range(KT):
            w = wqo.tile([128, C], BF16, tag="wqo", bufs=KT + NP,
                         name=f"wq{ct}")
            nc.sync.dma_start(w[:], wq_d[ct * 128:(ct + 1) * 128, :])
            wq_sb.append(w)
        for p in range(NP):
            w = wqo.tile([128, C], BF16, tag="wqo", bufs=KT + NP,
                         name=f"wo{p}")
            nc.sync.dma_start(w[:], wo_d[p * 128:(p + 1) * 128, :])
            wo_sb.append(w)

        ones2 = sb.tile([128, 2], BF16, tag="ones2", name="ones2")
        nc.gpsimd.memset(ones2[:], 1.0)
        onesks = sb.tile([128, 64], BF16, tag="onesks", name="onesks")
        nc.gpsimd.memset(onesks[:], 1.0)

        kvagg = sb.tile([128, NP * PSTR], F32, tag="kvagg", name="kvagg")
        nc.gpsimd.memset(kvagg[:], 0.0)

        qhat = [sb.tile([128, S], BF16, tag="qhat", bufs=NP, name=f"qhat{p}")
                for p in range(NP)]

        # ---- phase 1: k/v projections + kvT/ksum (kv one tile late) ----
        ktoks = [None] * TT
        vtoks = [None] * TT

        def emit_proj(tt):
            t0 = tt * 128
            xb = [xs_sb[ct][:, t0:t0 + 128] for ct in range(KT)]
            kp = ps.tile([128, C], F32, tag="pp", bufs=2, name=f"kp{tt}")
            for ch in range(2):
                for ct in range(KT):
                    nc.tensor.matmul(
                        kp[:, ch * 512:(ch + 1) * 512], xb[ct],
                        wk_sb[ct][:, ch * 512:(ch + 1) * 512],
                        start=(ct == 0), stop=(ct == KT - 1))
            vp = ps.tile([128, C], F32, tag="pp", bufs=2, name=f"vp{tt}")
            for ch in range(2):
                for ct in range(KT):
                    nc.tensor.matmul(
                        vp[:, ch * 512:(ch + 1) * 512], xb[ct],
                        wv_sb[ct][:, ch * 512:(ch + 1) * 512],
                        start=(ct == 0), stop=(ct == KT - 1))
            km = tr.tile([128, C], BF16, tag="km", bufs=2, name=f"km{tt}")
            nc.vector.tensor_scalar_min(km[:], kp[:], 0.0)
            ke = tr.tile([128, C], BF16, tag="ke", bufs=2, name=f"ke{tt}")
            nc.scalar.activation(ke[:], km[:], Exp)
            ktok = tr.tile([128, C], BF16, tag="ktok", bufs=3,
                           name=f"ktok{tt}")
            nc.vector.scalar_tensor_tensor(ktok[:], kp[:], 0.0, ke[:],
                                           Alu.max, Alu.add)
            vtok = tr.tile([128, C], BF16, tag="vtok", bufs=3,
                           name=f"vtok{tt}")
            nc.scalar.copy(vtok[:], vp[:])
            ktoks[tt], vtoks[tt] = ktok, vtok

        def emit_kv(tt):
            ktok, vtok = ktoks[tt], vtoks[tt]
            for g in range(3):
                p0, p1n = 3 * g, min(3 * g + 3, NP)
                kvt = ps.tile([128, (p1n - p0) * PSTR], F32, tag="kvt",
                              bufs=4, name=f"kvt{tt}_{g}",
                              padded_shape=[128, 512])
                for p in range(p0, p1n):
                    j = p - p0
                    nc.tensor.matmul(
                        kvt[:, j * PSTR:j * PSTR + 128],
                        vtok[:, p * 128:(p + 1) * 128],
                        ktok[:, p * 128:(p + 1) * 128],
                        start=True, stop=True)
                    nc.tensor.matmul(
                        kvt[:, j * PSTR + 128:j * PSTR + 130],
                        ktok[:, p * 128:(p + 1) * 128],
                        ones2[:], start=True, stop=True)
                nc.vector.tensor_add(
                    kvagg[:, p0 * PSTR:p1n * PSTR],
                    kvagg[:, p0 * PSTR:p1n * PSTR], kvt[:])

        for tt in range(TT):
            emit_proj(tt)
            if tt > 0:
                emit_kv(tt - 1)
        emit_kv(TT - 1)

        # ---- pair AllReduce (overlaps phase 1.5) -----------------------
        bounce_in = dram.tile([128, NP * PSTR], F32, name="bounce_in")
        bounce_out = dram.tile([128, NP * PSTR], F32, name="bounce_out")
        nc.sync.dma_start(bounce_in[:], kvagg[:])
        nc.gpsimd.collective_compute(
            "AllReduce", Alu.add,
            ins=[bounce_in.opt()], outs=[bounce_out.opt()],
            replica_groups=[[2 * i, 2 * i + 1] for i in range(N_CORES // 2)])
        kvcoll = sb.tile([128, NP * PSTR], F32, tag="kvcoll", name="kvcoll")
        nc.sync.dma_start(kvcoll[:], bounce_out[:])

        # ---- unpack on Pool: runs during phase 1.5 ---------------------
        kvbs, KS = [], []
        for p in range(NP):
            c0 = p * PSTR
            kvb = sb.tile([128, 128], BF16, tag="kvb", bufs=NP,
                          name=f"kvb{p}")
            nc.gpsimd.memset(kvb[:], 0.0)
            nc.gpsimd.tensor_copy(kvb[0:64, 0:64],
                                  kvcoll[0:64, c0:c0 + 64])
            nc.gpsimd.tensor_copy(kvb[64:128, 64:128],
                                  kvcoll[64:128, c0 + 64:c0 + 128])
            kvbs.append(kvb)
            ks = sb.tile([128, 128], BF16, tag="KS", bufs=NP, name=f"KS{p}")
            nc.gpsimd.memset(ks[:], 0.0)
            nc.gpsimd.tensor_scalar_mul(
                ks[0:64, 0:64], onesks[0:64, :],
                kvcoll[0:64, c0 + 128:c0 + 129])
            nc.gpsimd.tensor_scalar_mul(
                ks[64:128, 64:128], onesks[64:128, :],
                kvcoll[64:128, c0 + 128:c0 + 129])
            KS.append(ks)

        # ---- phase 1.5: q projection + ELU -----------------------------
        for p in range(NP):
            for hh in range(2):
                h0 = hh * 1024
                qp = ps.tile([128, 1024], F32, tag="pp", bufs=2,
                             name=f"qp{p}_{hh}")
                for chk in range(2):
                    for ct in range(KT):
                        nc.tensor.matmul(
                            qp[:, chk * 512:(chk + 1) * 512],
                            wq_sb[ct][:, p * 128:(p + 1) * 128],
                            xs_sb[ct][:, h0 + chk * 512:
                                       h0 + (chk + 1) * 512],
                            start=(ct == 0), stop=(ct == KT - 1))
                qm = tr.tile([128, 1024], BF16, tag="qm", bufs=2,
                             name=f"qm{p}_{hh}")
                nc.vector.tensor_scalar_min(qm[:], qp[:], 0.0)
                qe = tr.tile([128, 1024], BF16, tag="qe", bufs=2,
                             name=f"qe{p}_{hh}")
                nc.scalar.activation(qe[:], qm[:], Exp)
                nc.vector.scalar_tensor_tensor(
                    qhat[p][:, h0:h0 + 1024], qp[:], 0.0, qe[:],
                    Alu.max, Alu.add)

        # ---- phase 2: G, denominators, scale, output projection --------
        G = []
        for p in range(NP):
            gt = sb.tile([128, C], BF16, tag="G", bufs=NP, name=f"G{p}")
            for ch in range(2):
                gp = ps.tile([128, 512], F32, tag="kvt", bufs=4,
                             name=f"gp{p}_{ch}")
                nc.tensor.matmul(gp[:], kvbs[p][:],
                                 wo_sb[p][:, ch * 512:(ch + 1) * 512],
                                 start=True, stop=True)
                nc.scalar.copy(gt[:, ch * 512:(ch + 1) * 512], gp[:])
            G.append(gt)

        def emit_scale(p, chk):
            qsl = qhat[p][:, chk * 512:(chk + 1) * 512]
            dnb = ps.tile([128, 512], F32, tag="kvt", bufs=4,
                          name=f"dnb{p}_{chk}")
            nc.tensor.matmul(dnb[:], KS[p][:], qsl, start=True, stop=True)
            rpb = tr.tile([128, 512], BF16, tag="rpb", bufs=3,
                          name=f"rpb{p}_{chk}")
            with nc.allow_low_precision(reason="recip of denom"):
                nc.vector.reciprocal(rpb[:], dnb[:])
            nc.gpsimd.tensor_mul(qsl, qsl, rpb[:])

        for p in range(NP):
            emit_scale(p, 0)
        for chk in range(S // 512):
            groups = [(mt, ch) for mt in range(chk * 4, chk * 4 + 4)
                      for ch in range(2)]
            for i, (mt, ch) in enumerate(groups):
                if chk + 1 < S // 512 and i < NP:
                    emit_scale(i, chk + 1)
                r0 = mt * 128
                yp = ps.tile([128, 512], F32, tag="pp", bufs=2,
                             name=f"yp{mt}_{ch}")
                for p in range(NP):
                    nc.tensor.matmul(
                        yp[:], qhat[p][:, r0:r0 + 128],
                        G[p][:, ch * 512:(ch + 1) * 512],
                        start=(p == 0), stop=(p == NP - 1))
                ysb = tr.tile([128, 512], F32, tag="ysb", bufs=3,
                              name=f"ysb{mt}_{ch}")
                nc.scalar.copy(ysb[:], yp[:])
                nc.sync.dma_start(
                    out_d[r0:r0 + 128, ch * 512:(ch + 1) * 512], ysb[:])


def _build(has_bias: bool):
    KT = 9 if has_bias else 8
    KC = KT * 128

    nc = bacc.Bacc("TRN2", target_bir_lowering=False, debug=False,
                   num_devices=N_CORES)
    xt_d = nc.dram_tensor("xt", [KC, S], BF16, kind="ExternalInput").ap()
    wk_d = nc.dram_tensor("wkt", [KC, C], BF16, kind="ExternalInput").ap()
    wv_d = nc.dram_tensor("wvt", [KC, C], BF16, kind="ExternalInput").ap()
    wq_d = nc.dram_tensor("wqt", [KC, C], BF16, kind="ExternalInput").ap()
    wo_d = nc.dram_tensor("wot", [KC, C], BF16, kind="ExternalInput").ap()
    out_d = nc.dram_tensor("out", [S, C], F32, kind="ExternalOutput").ap()

    with tile.TileContext(nc) as tc:
        _emit(nc, tc, KT, xt_d, wk_d, wv_d, wq_d, wo_d, out_d)
    nc.compile()
    return nc


def _prep_host(inputs, KT):
    """Host-side shard + transpose prep. Returns in_maps for the 8 cores."""
    KC = KT * 128
    npdt = mybir.dt.np(BF16)
    x = np.asarray(inputs["x"], np.float32).reshape(B * T, C)

    def padw(w, b):
        wt = np.ascontiguousarray(np.asarray(w, np.float32).T)  # [Cin, Cout]
        if KC == C:
            return wt.astype(npdt)
        out = np.zeros((KC, C), np.float32)
        out[:C] = wt
        out[C] = np.asarray(b, np.float32)
        return out.astype(npdt)

    wkt = padw(inputs["Wk"], inputs["bk"])
    wvt = padw(inputs["Wv"], inputs["bv"])
    wqt = padw(inputs["Wq"], inputs["bq"])
    wot = padw(inputs["Wo"], np.zeros(C))   # bo applied on host

    in_maps = []
    for c in range(N_CORES):
        sh = x[c * S:(c + 1) * S]
        xt = np.zeros((KC, S), np.float32)
        xt[:C] = sh.T
        if KC > C:
            xt[C] = 1.0
        in_maps.append({
            "xt": np.ascontiguousarray(xt.astype(npdt)),
            "wkt": wkt, "wvt": wvt, "wqt": wqt, "wot": wot,
        })
    return in_maps


def _get_nc(has_bias):
    if has_bias not in _cache:
        _cache[has_bias] = _build(has_bias)
    return _cache[has_bias]


def kernel(**inputs):
    assert np.asarray(inputs["x"]).shape == (B, T, C)
    has_bias = any(
        np.any(np.asarray(inputs[k])) for k in ("bq", "bk", "bv"))
    nc = _get_nc(has_bias)
    in_maps = _prep_host(inputs, 9 if has_bias else 8)
    res = bass_utils.run_bass_kernel_spmd(
        nc, in_maps, core_ids=list(range(N_CORES)))
    y = np.concatenate(
        [res.results[c]["out"] for c in range(N_CORES)], axis=0)
    y = y.reshape(B, T, C).astype(np.float32)
    bo = np.asarray(inputs["bo"], np.float32)
    if np.any(bo):
        y = y + bo
    return y


# revision 3
# speedup vs baseline: 1.0460x; 1.0080x over previous
"""Linear attention (ELU+1 feature map) on 8 TRN2 NeuronCores — v7.

Algorithm (see v2): bf16 projections; ELU+1 split DVE/ACT/DVE; per-pair
kvT = vtok^T @ ktok and ksum = ktok^T @ ones2 single-shot into PSUM,
DVE-accumulated in f32 (PSUM accumulation groups interleaved within a bank
are broken on HW); pair AllReduce of kv/ksum overlapped with the q
projection; G_p = kv_p @ Wo_p^T folds kv into the output projection;
KS_p (half-masked free-broadcast ksum) gives the denominator already
broadcast to 128 partitions in ONE matmul; qs = qhat * (1/dnB) in place
(DVE reciprocal + Pool multiply); out = qs^T @ G accumulated over pairs.

Scheduling (v5):
  - ONE PSUM pool for the whole kernel. Opening a new pool makes every new
    tile wait on ALL users of the closed pool (release-boundary barrier,
    measured 5.8us at each phase edge); instead phase 1.5's qp reuses the
    kp/vp tag, G/dnB reuse the kvt tag, yp reuses the pp tag, so cross-
    phase waits are per-slot and land exactly on the pipelined tail.
  - ONE transient SBUF pool for the same reason.
  - kv/ksum matmuls of token-tile tt-1 are emitted after the projections
    of tt; kvt has 4 PSUM bufs so the g2 matmul never waits on the DVE
    aggregation adds.
  - input DMAs ride HWDGE queues (sync for wk, scalar for the rest) in
    consumption order -- gpsimd SWDGE DMAs would occupy the Pool engine.
  - phase-2 unpack runs on Pool during phase 1.5; chunk c+1's
    denominator chains interleave with chunk c's output projection.
"""

import sys
import numpy as np

for _p in ("/opt/trn_rl_repo", "/opt/pypackages"):
    if _p not in sys.path:
        sys.path.append(_p)

import concourse.bacc as bacc
import concourse.mybir as mybir
import concourse.tile as tile
from concourse import bass_utils

F32 = mybir.dt.float32
BF16 = mybir.dt.bfloat16
ACTF = mybir.ActivationFunctionType
Alu = mybir.AluOpType

N_CORES = 8
B, T, C = 4, 4096, 1024
H, D = 16, 64
S = B * T // N_CORES          # 2048 tokens per core
NP = 8                        # head pairs (128 channels each)
TT = S // 128                 # 16 token tiles per core
PSTR = 130                    # kv slot: 128 kvT cols + 2 ksum cols
XCH = 512                     # xs DMA token chunk
OUT_DT = BF16                 # device output dtype (host converts to f32)

_cache = {}


def _emit(nc, tc, KT, xt_d, wk_d, wv_d, wq_d, wo_d, out_d):
    Exp = ACTF.Exp

    with (
        tc.tile_pool(name="wkv", bufs=1) as wkv,
        tc.tile_pool(name="wqo", bufs=1) as wqo,
        tc.tile_pool(name="persist", bufs=1) as sb,
        tc.tile_pool(name="trans", bufs=1) as tr,
        tc.tile_pool(name="psum", bufs=1, space="PSUM") as ps,
        tc.tile_pool(name="dram", bufs=1, space="DRAM") as dram,
    ):
        # ---- input DMAs, in consumption order --------------------------
        # sync: wk (gates the very first matmuls); scalar: everything else
        wk_sb, wv_sb = [], []
        for ct in range(KT):
            w = wkv.tile([128, C], BF16, tag="wkv", bufs=2 * KT,
                         name=f"wk{ct}")
            nc.sync.dma_start(w[:], wk_d[ct * 128:(ct + 1) * 128, :])
            wk_sb.append(w)

        xs_sb = [sb.tile([128, S], BF16, tag="xs", bufs=KT, name=f"xs{ct}")
                 for ct in range(KT)]
        for ct in range(KT):
            nc.scalar.dma_start(xs_sb[ct][:, 0:XCH],
                                xt_d[ct * 128:(ct + 1) * 128, 0:XCH])
        for ct in range(KT):
            w = wkv.tile([128, C], BF16, tag="wkv", bufs=2 * KT,
                         name=f"wv{ct}")
            nc.sync.dma_start(w[:], wv_d[ct * 128:(ct + 1) * 128, :])
            wv_sb.append(w)
        for ct in range(KT):
            nc.sync.dma_start(xs_sb[ct][:, XCH:S],
                              xt_d[ct * 128:(ct + 1) * 128, XCH:S])
        wq_sb, wo_sb = [], []
        for ct in range(KT):
            w = wqo.tile([128, C], BF16, tag="wqo", bufs=KT + NP,
                         name=f"wq{ct}")
            nc.sync.dma_start(w[:], wq_d[ct * 128:(ct + 1) * 128, :])
            wq_sb.append(w)
        for p in range(NP):
            w = wqo.tile([128, C], BF16, tag="wqo", bufs=KT + NP,
                         name=f"wo{p}")
            nc.sync.dma_start(w[:], wo_d[p * 128:(p + 1) * 128, :])
            wo_sb.append(w)

        ones2 = sb.tile([128, 2], BF16, tag="ones2", name="ones2")
        nc.gpsimd.memset(ones2[:], 1.0)
        onesks = sb.tile([128, 64], BF16, tag="onesks", name="onesks")
        nc.gpsimd.memset(onesks[:], 1.0)

        kvagg = sb.tile([128, NP * PSTR], F32, tag="kvagg", name="kvagg")
        nc.gpsimd.memset(kvagg[:], 0.0)

        qhat = [sb.tile([128, S], BF16, tag="qhat", bufs=NP, name=f"qhat{p}")
                for p in range(NP)]

        # ---- phase 1: k/v projections + kvT/ksum (kv one tile late) ----
        ktoks = [None] * TT
        vtoks = [None] * TT

        def emit_proj(tt):
            t0 = tt * 128
            xb = [xs_sb[ct][:, t0:t0 + 128] for ct in range(KT)]
            kp = ps.tile([128, C], F32, tag="pp", bufs=2, name=f"kp{tt}")
            for ch in range(2):
                for ct in range(KT):
                    nc.tensor.matmul(
                        kp[:, ch * 512:(ch + 1) * 512], xb[ct],
                        wk_sb[ct][:, ch * 512:(ch + 1) * 512],
                        start=(ct == 0), stop=(ct == KT - 1))
            vp = ps.tile([128, C], F32, tag="pp", bufs=2, name=f"vp{tt}")
            for ch in range(2):
                for ct in range(KT):
                    nc.tensor.matmul(
                        vp[:, ch * 512:(ch + 1) * 512], xb[ct],
                        wv_sb[ct][:, ch * 512:(ch + 1) * 512],
                        start=(ct == 0), stop=(ct == KT - 1))
            km = tr.tile([128, C], BF16, tag="km", bufs=2, name=f"km{tt}")
            ke = tr.tile([128, C], BF16, tag="ke", bufs=2, name=f"ke{tt}")
            ktok = tr.tile([128, C], BF16, tag="ktok", bufs=3,
                           name=f"ktok{tt}")
            HS = (slice(0, 512), slice(512, 1024))
            for h in HS:
                nc.vector.tensor_scalar_min(km[:, h], kp[:, h], 0.0)
            for h in HS:
                nc.scalar.activation(ke[:, h], km[:, h], Exp)
            for h in HS:
                nc.vector.scalar_tensor_tensor(ktok[:, h], kp[:, h], 0.0,
                                               ke[:, h], Alu.max, Alu.add)
            vtok = tr.tile([128, C], BF16, tag="vtok", bufs=3,
                           name=f"vtok{tt}")
            nc.scalar.copy(vtok[:], vp[:])
            ktoks[tt], vtoks[tt] = ktok, vtok

        def emit_kv(tt):
            ktok, vtok = ktoks[tt], vtoks[tt]
            for g in range(3):
                p0, p1n = 3 * g, min(3 * g + 3, NP)
                kvt = ps.tile([128, (p1n - p0) * PSTR], F32, tag="kvt",
                              bufs=4, name=f"kvt{tt}_{g}",
                              padded_shape=[128, 512])
                for p in range(p0, p1n):
                    j = p - p0
                    nc.tensor.matmul(
                        kvt[:, j * PSTR:j * PSTR + 128],
                        vtok[:, p * 128:(p + 1) * 128],
                        ktok[:, p * 128:(p + 1) * 128],
                        start=True, stop=True)
                    nc.tensor.matmul(
                        kvt[:, j * PSTR + 128:j * PSTR + 130],
                        ktok[:, p * 128:(p + 1) * 128],
                        ones2[:], start=True, stop=True)
                nc.vector.tensor_add(
                    kvagg[:, p0 * PSTR:p1n * PSTR],
                    kvagg[:, p0 * PSTR:p1n * PSTR], kvt[:])

        for tt in range(TT):
            emit_proj(tt)
            if tt > 0:
                emit_kv(tt - 1)
        emit_kv(TT - 1)

        # ---- pair AllReduce (overlaps phase 1.5) -----------------------
        bounce_in = dram.tile([128, NP * PSTR], F32, name="bounce_in")
        bounce_out = dram.tile([128, NP * PSTR], F32, name="bounce_out")
        nc.sync.dma_start(bounce_in[:], kvagg[:])
        nc.gpsimd.collective_compute(
            "AllReduce", Alu.add,
            ins=[bounce_in.opt()], outs=[bounce_out.opt()],
            replica_groups=[[2 * i, 2 * i + 1] for i in range(N_CORES // 2)])
        kvcoll = sb.tile([128, NP * PSTR], F32, tag="kvcoll", name="kvcoll")
        nc.sync.dma_start(kvcoll[:], bounce_out[:])

        # ---- unpack on Pool: runs during phase 1.5 ---------------------
        kvbs, KS = [], []
        for p in range(NP):
            c0 = p * PSTR
            kvb = sb.tile([128, 128], BF16, tag="kvb", bufs=NP,
                          name=f"kvb{p}")
            nc.gpsimd.memset(kvb[:], 0.0)
            nc.gpsimd.tensor_copy(kvb[0:64, 0:64],
                                  kvcoll[0:64, c0:c0 + 64])
            nc.gpsimd.tensor_copy(kvb[64:128, 64:128],
                                  kvcoll[64:128, c0 + 64:c0 + 128])
            kvbs.append(kvb)
            ks = sb.tile([128, 128], BF16, tag="KS", bufs=NP, name=f"KS{p}")
            nc.gpsimd.memset(ks[:], 0.0)
            nc.gpsimd.tensor_scalar_mul(
                ks[0:64, 0:64], onesks[0:64, :],
                kvcoll[0:64, c0 + 128:c0 + 129])
            nc.gpsimd.tensor_scalar_mul(
                ks[64:128, 64:128], onesks[64:128, :],
                kvcoll[64:128, c0 + 128:c0 + 129])
            KS.append(ks)

        # qs = qhat * 1/(KS^T qhat): denominator matmul, reciprocal,
        # in-place Pool multiply. Chunk-0 chains are pre-emitted inside the
        # phase-1.5 tail so the output projection starts immediately.
        def emit_scale(p, chk):
            qsl = qhat[p][:, chk * 512:(chk + 1) * 512]
            dnb = ps.tile([128, 512], F32, tag="kvt", bufs=4,
                          name=f"dnb{p}_{chk}")
            nc.tensor.matmul(dnb[:], KS[p][:], qsl, start=True, stop=True)
            rpb = tr.tile([128, 512], BF16, tag="rpb", bufs=3,
                          name=f"rpb{p}_{chk}")
            with nc.allow_low_precision(reason="recip of denom"):
                nc.vector.reciprocal(rpb[:], dnb[:])
            nc.gpsimd.tensor_mul(qsl, qsl, rpb[:])

        G = [sb.tile([128, C], BF16, tag="G", bufs=NP, name=f"G{p}")
             for p in range(NP)]

        def emit_g(p):
            for ch in range(2):
                gp = ps.tile([128, 512], F32, tag="kvt", bufs=4,
                             name=f"gp{p}_{ch}")
                nc.tensor.matmul(gp[:], kvbs[p][:],
                                 wo_sb[p][:, ch * 512:(ch + 1) * 512],
                                 start=True, stop=True)
                nc.scalar.copy(G[p][:, ch * 512:(ch + 1) * 512], gp[:])

        # ---- phase 1.5: q projection + ELU, with the G build and the
        # chunk-0 denominator chains interleaved (their inputs are ready
        # once the AllReduce lands mid-phase) ------------------------------
        for p in range(NP):
            if p >= 4:
                emit_g(2 * (p - 4))
                emit_g(2 * (p - 4) + 1)
            if p >= 2:
                emit_scale(p - 2, 0)
            for hh in range(2):
                h0 = hh * 1024
                qp = ps.tile([128, 1024], F32, tag="pp", bufs=2,
                             name=f"qp{p}_{hh}")
                for chk in range(2):
                    for ct in range(KT):
                        nc.tensor.matmul(
                            qp[:, chk * 512:(chk + 1) * 512],
                            wq_sb[ct][:, p * 128:(p + 1) * 128],
                            xs_sb[ct][:, h0 + chk * 512:
                                       h0 + (chk + 1) * 512],
                            start=(ct == 0), stop=(ct == KT - 1))
                qm = tr.tile([128, 1024], BF16, tag="qm", bufs=2,
                             name=f"qm{p}_{hh}")
                qe = tr.tile([128, 1024], BF16, tag="qe", bufs=2,
                             name=f"qe{p}_{hh}")
                HS = (slice(0, 512), slice(512, 1024))
                for hs in HS:
                    nc.vector.tensor_scalar_min(qm[:, hs], qp[:, hs], 0.0)
                for hs in HS:
                    nc.scalar.activation(qe[:, hs], qm[:, hs], Exp)
                for hs in HS:
                    nc.vector.scalar_tensor_tensor(
                        qhat[p][:, h0 + hs.start:h0 + hs.stop], qp[:, hs],
                        0.0, qe[:, hs], Alu.max, Alu.add)

        # ---- phase 2: remaining denominators + output projection -------

        for p in range(NP - 2, NP):
            emit_scale(p, 0)
        for chk in range(S // 512):
            groups = [(mt, ch) for mt in range(chk * 4, chk * 4 + 4)
                      for ch in range(2)]
            for i, (mt, ch) in enumerate(groups):
                if chk + 1 < S // 512 and i < NP:
                    emit_scale(i, chk + 1)
                r0 = mt * 128
                yp = ps.tile([128, 512], F32, tag="pp", bufs=2,
                             name=f"yp{mt}_{ch}")
                for p in range(NP):
                    nc.tensor.matmul(
                        yp[:], qhat[p][:, r0:r0 + 128],
                        G[p][:, ch * 512:(ch + 1) * 512],
                        start=(p == 0), stop=(p == NP - 1))
                ysb = tr.tile([128, 512], BF16, tag="ysb", bufs=3,
                              name=f"ysb{mt}_{ch}")
                nc.scalar.copy(ysb[:], yp[:])
                nc.sync.dma_start(
                    out_d[r0:r0 + 128, ch * 512:(ch + 1) * 512], ysb[:])


def _build(has_bias: bool):
    KT = 9 if has_bias else 8
    KC = KT * 128

    nc = bacc.Bacc("TRN2", target_bir_lowering=False, debug=False,
                   num_devices=N_CORES)
    xt_d = nc.dram_tensor("xt", [KC, S], BF16, kind="ExternalInput").ap()
    wk_d = nc.dram_tensor("wkt", [KC, C], BF16, kind="ExternalInput").ap()
    wv_d = nc.dram_tensor("wvt", [KC, C], BF16, kind="ExternalInput").ap()
    wq_d = nc.dram_tensor("wqt", [KC, C], BF16, kind="ExternalInput").ap()
    wo_d = nc.dram_tensor("wot", [KC, C], BF16, kind="ExternalInput").ap()
    out_d = nc.dram_tensor("out", [S, C], BF16, kind="ExternalOutput").ap()

    with tile.TileContext(nc) as tc:
        _emit(nc, tc, KT, xt_d, wk_d, wv_d, wq_d, wo_d, out_d)
    nc.compile()
    return nc


def _prep_host(inputs, KT):
    """Host-side shard + transpose prep. Returns in_maps for the 8 cores."""
    KC = KT * 128
    npdt = mybir.dt.np(BF16)
    x = np.asarray(inputs["x"], np.float32).reshape(B * T, C)

    def padw(w, b):
        wt = np.ascontiguousarray(np.asarray(w, np.float32).T)  # [Cin, Cout]
        if KC == C:
            return wt.astype(npdt)
        out = np.zeros((KC, C), np.float32)
        out[:C] = wt
        out[C] = np.asarray(b, np.float32)
        return out.astype(npdt)

    wkt = padw(inputs["Wk"], inputs["bk"])
    wvt = padw(inputs["Wv"], inputs["bv"])
    wqt = padw(inputs["Wq"], inputs["bq"])
    wot = padw(inputs["Wo"], np.zeros(C))   # bo applied on host

    in_maps = []
    for c in range(N_CORES):
        sh = x[c * S:(c + 1) * S]
        xt = np.zeros((KC, S), np.float32)
        xt[:C] = sh.T
        if KC > C:
            xt[C] = 1.0
        in_maps.append({
            "xt": np.ascontiguousarray(xt.astype(npdt)),
            "wkt": wkt, "wvt": wvt, "wqt": wqt, "wot": wot,
        })
    return in_maps


def _get_nc(has_bias):
    if has_bias not in _cache:
        _cache[has_bias] = _build(has_bias)
    return _cache[has_bias]


def kernel(**inputs):
    assert np.asarray(inputs["x"]).shape == (B, T, C)
    has_bias = any(
        np.any(np.asarray(inputs[k])) for k in ("bq", "bk", "bv"))
    nc = _get_nc(has_bias)
    in_maps = _prep_host(inputs, 9 if has_bias else 8)
    res = bass_utils.run_bass_kernel_spmd(
        nc, in_maps, core_ids=list(range(N_CORES)))
    y = np.concatenate(
        [np.asarray(res.results[c]["out"], np.float32)
         for c in range(N_CORES)], axis=0)
    y = y.reshape(B, T, C)
    bo = np.asarray(inputs["bo"], np.float32)
    if np.any(bo):
        y = y + bo
    return y


# revision 4
# speedup vs baseline: 1.0503x; 1.0042x over previous
"""Linear attention (ELU+1 feature map) on 8 TRN2 NeuronCores — v10.

Algorithm (see v2): bf16 projections; ELU+1 split DVE/ACT/DVE; per-pair
kvT = vtok^T @ ktok and ksum = ktok^T @ ones2 single-shot into PSUM,
DVE-accumulated in f32 (PSUM accumulation groups interleaved within a bank
are broken on HW); pair AllReduce of kv/ksum overlapped with the q
projection; G_p = kv_p @ Wo_p^T folds kv into the output projection;
KS_p (half-masked free-broadcast ksum) gives the denominator already
broadcast to 128 partitions in ONE matmul; qs = qhat * (1/dnB) in place
(DVE reciprocal + Pool multiply); out = qs^T @ G accumulated over pairs.

Scheduling (v5):
  - ONE PSUM pool for the whole kernel. Opening a new pool makes every new
    tile wait on ALL users of the closed pool (release-boundary barrier,
    measured 5.8us at each phase edge); instead phase 1.5's qp reuses the
    kp/vp tag, G/dnB reuse the kvt tag, yp reuses the pp tag, so cross-
    phase waits are per-slot and land exactly on the pipelined tail.
  - ONE transient SBUF pool for the same reason.
  - kv/ksum matmuls of token-tile tt-1 are emitted after the projections
    of tt; kvt has 4 PSUM bufs so the g2 matmul never waits on the DVE
    aggregation adds.
  - input DMAs ride HWDGE queues (sync for wk, scalar for the rest) in
    consumption order -- gpsimd SWDGE DMAs would occupy the Pool engine.
  - phase-2 unpack runs on Pool during phase 1.5; chunk c+1's
    denominator chains interleave with chunk c's output projection.
"""

import sys
import numpy as np

for _p in ("/opt/trn_rl_repo", "/opt/pypackages"):
    if _p not in sys.path:
        sys.path.append(_p)

import concourse.bacc as bacc
import concourse.mybir as mybir
import concourse.tile as tile
from concourse import bass_utils

F32 = mybir.dt.float32
BF16 = mybir.dt.bfloat16
ACTF = mybir.ActivationFunctionType
Alu = mybir.AluOpType

N_CORES = 8
B, T, C = 4, 4096, 1024
H, D = 16, 64
S = B * T // N_CORES          # 2048 tokens per core
NP = 8                        # head pairs (128 channels each)
TT = S // 128                 # 16 token tiles per core
PSTR = 130                    # kv slot: 128 kvT cols + 2 ksum cols
XCH = 512                     # xs DMA token chunk
OUT_DT = BF16                 # device output dtype (host converts to f32)

_cache = {}


def _emit(nc, tc, KT, xt_d, wk_d, wv_d, wq_d, wo_d, out_d):
    Exp = ACTF.Exp

    with (
        tc.tile_pool(name="wkv", bufs=1) as wkv,
        tc.tile_pool(name="wqo", bufs=1) as wqo,
        tc.tile_pool(name="persist", bufs=1) as sb,
        tc.tile_pool(name="trans", bufs=1) as tr,
        tc.tile_pool(name="psum", bufs=1, space="PSUM") as ps,
        tc.tile_pool(name="dram", bufs=1, space="DRAM") as dram,
    ):
        # ---- input DMAs, in consumption order --------------------------
        # sync: wk (gates the very first matmuls); scalar: everything else
        wk_sb, wv_sb = [], []
        for ct in range(KT):
            w = wkv.tile([128, C], BF16, tag="wkv", bufs=2 * KT,
                         name=f"wk{ct}")
            nc.sync.dma_start(w[:], wk_d[ct * 128:(ct + 1) * 128, :])
            wk_sb.append(w)

        xs_sb = [sb.tile([128, S], BF16, tag="xs", bufs=KT, name=f"xs{ct}")
                 for ct in range(KT)]
        for ct in range(KT):
            nc.scalar.dma_start(xs_sb[ct][:, 0:XCH],
                                xt_d[ct * 128:(ct + 1) * 128, 0:XCH])
        for ct in range(KT):
            w = wkv.tile([128, C], BF16, tag="wkv", bufs=2 * KT,
                         name=f"wv{ct}")
            nc.sync.dma_start(w[:], wv_d[ct * 128:(ct + 1) * 128, :])
            wv_sb.append(w)
        for ct in range(KT):
            nc.sync.dma_start(xs_sb[ct][:, XCH:S],
                              xt_d[ct * 128:(ct + 1) * 128, XCH:S])
        wq_sb, wo_sb = [], []
        for ct in range(KT):
            w = wqo.tile([128, C], BF16, tag="wqo", bufs=KT + NP,
                         name=f"wq{ct}")
            nc.sync.dma_start(w[:], wq_d[ct * 128:(ct + 1) * 128, :])
            wq_sb.append(w)
        for p in range(NP):
            w = wqo.tile([128, C], BF16, tag="wqo", bufs=KT + NP,
                         name=f"wo{p}")
            nc.sync.dma_start(w[:], wo_d[p * 128:(p + 1) * 128, :])
            wo_sb.append(w)

        ones2 = sb.tile([128, 2], BF16, tag="ones2", name="ones2")
        nc.gpsimd.memset(ones2[:], 1.0)
        onesks = sb.tile([128, 64], BF16, tag="onesks", name="onesks")
        nc.gpsimd.memset(onesks[:], 1.0)

        kvagg = sb.tile([128, NP * PSTR], F32, tag="kvagg", name="kvagg")
        nc.gpsimd.memset(kvagg[:], 0.0)

        qhat = [sb.tile([128, S], BF16, tag="qhat", bufs=NP, name=f"qhat{p}")
                for p in range(NP)]

        # ---- phase 1: k/v projections + kvT/ksum (kv one tile late) ----
        ktoks = [None] * TT
        vtoks = [None] * TT

        def emit_proj(tt):
            t0 = tt * 128
            xb = [xs_sb[ct][:, t0:t0 + 128] for ct in range(KT)]
            kp = ps.tile([128, C], F32, tag="pp", bufs=2, name=f"kp{tt}")
            for ct in range(KT):       # ct-major: arrival-paced at startup
                for ch in range(2):
                    nc.tensor.matmul(
                        kp[:, ch * 512:(ch + 1) * 512], xb[ct],
                        wk_sb[ct][:, ch * 512:(ch + 1) * 512],
                        start=(ct == 0), stop=(ct == KT - 1))
            vp = ps.tile([128, C], F32, tag="pp", bufs=2, name=f"vp{tt}")
            for ct in range(KT):
                for ch in range(2):
                    nc.tensor.matmul(
                        vp[:, ch * 512:(ch + 1) * 512], xb[ct],
                        wv_sb[ct][:, ch * 512:(ch + 1) * 512],
                        start=(ct == 0), stop=(ct == KT - 1))
            km = tr.tile([128, C], BF16, tag="km", bufs=2, name=f"km{tt}")
            ke = tr.tile([128, C], BF16, tag="ke", bufs=2, name=f"ke{tt}")
            ktok = tr.tile([128, C], BF16, tag="ktok", bufs=3,
                           name=f"ktok{tt}")
            HS = (slice(0, 512), slice(512, 1024))
            for h in HS:
                nc.vector.tensor_scalar_min(km[:, h], kp[:, h], 0.0)
            for h in HS:
                nc.scalar.activation(ke[:, h], km[:, h], Exp)
            for h in HS:
                nc.vector.scalar_tensor_tensor(ktok[:, h], kp[:, h], 0.0,
                                               ke[:, h], Alu.max, Alu.add)
            vtok = tr.tile([128, C], BF16, tag="vtok", bufs=3,
                           name=f"vtok{tt}")
            nc.scalar.copy(vtok[:], vp[:])
            ktoks[tt], vtoks[tt] = ktok, vtok

        def emit_kv(tt):
            ktok, vtok = ktoks[tt], vtoks[tt]
            for g in range(3):
                p0, p1n = 3 * g, min(3 * g + 3, NP)
                kvt = ps.tile([128, (p1n - p0) * PSTR], F32, tag="kvt",
                              bufs=4, name=f"kvt{tt}_{g}",
                              padded_shape=[128, 512])
                for p in range(p0, p1n):
                    j = p - p0
                    nc.tensor.matmul(
                        kvt[:, j * PSTR:j * PSTR + 128],
                        vtok[:, p * 128:(p + 1) * 128],
                        ktok[:, p * 128:(p + 1) * 128],
                        start=True, stop=True)
                    nc.tensor.matmul(
                        kvt[:, j * PSTR + 128:j * PSTR + 130],
                        ktok[:, p * 128:(p + 1) * 128],
                        ones2[:], start=True, stop=True)
                nc.vector.tensor_add(
                    kvagg[:, p0 * PSTR:p1n * PSTR],
                    kvagg[:, p0 * PSTR:p1n * PSTR], kvt[:])

        for tt in range(TT):
            emit_proj(tt)
            if tt > 0:
                emit_kv(tt - 1)
        emit_kv(TT - 1)

        # ---- pair AllReduce (overlaps phase 1.5) -----------------------
        bounce_in = dram.tile([128, NP * PSTR], F32, name="bounce_in")
        bounce_out = dram.tile([128, NP * PSTR], F32, name="bounce_out")
        nc.sync.dma_start(bounce_in[:], kvagg[:])
        nc.gpsimd.collective_compute(
            "AllReduce", Alu.add,
            ins=[bounce_in.opt()], outs=[bounce_out.opt()],
            replica_groups=[[2 * i, 2 * i + 1] for i in range(N_CORES // 2)])
        kvcoll = sb.tile([128, NP * PSTR], F32, tag="kvcoll", name="kvcoll")
        nc.sync.dma_start(kvcoll[:], bounce_out[:])

        # ---- unpack on Pool: runs during phase 1.5 ---------------------
        kvbs, KS = [], []
        for p in range(NP):
            c0 = p * PSTR
            kvb = sb.tile([128, 128], BF16, tag="kvb", bufs=NP,
                          name=f"kvb{p}")
            nc.gpsimd.memset(kvb[:], 0.0)
            nc.gpsimd.tensor_copy(kvb[0:64, 0:64],
                                  kvcoll[0:64, c0:c0 + 64])
            nc.gpsimd.tensor_copy(kvb[64:128, 64:128],
                                  kvcoll[64:128, c0 + 64:c0 + 128])
            kvbs.append(kvb)
            ks = sb.tile([128, 128], BF16, tag="KS", bufs=NP, name=f"KS{p}")
            nc.gpsimd.memset(ks[:], 0.0)
            nc.gpsimd.tensor_scalar_mul(
                ks[0:64, 0:64], onesks[0:64, :],
                kvcoll[0:64, c0 + 128:c0 + 129])
            nc.gpsimd.tensor_scalar_mul(
                ks[64:128, 64:128], onesks[64:128, :],
                kvcoll[64:128, c0 + 128:c0 + 129])
            KS.append(ks)

        # qs = qhat * 1/(KS^T qhat): denominator matmul, reciprocal,
        # in-place Pool multiply. Chunk-0 chains are pre-emitted inside the
        # phase-1.5 tail so the output projection starts immediately.
        def emit_scale(p, chk):
            qsl = qhat[p][:, chk * 512:(chk + 1) * 512]
            dnb = ps.tile([128, 512], F32, tag="kvt", bufs=4,
                          name=f"dnb{p}_{chk}")
            nc.tensor.matmul(dnb[:], KS[p][:], qsl, start=True, stop=True)
            rpb = tr.tile([128, 512], BF16, tag="rpb", bufs=3,
                          name=f"rpb{p}_{chk}")
            with nc.allow_low_precision(reason="recip of denom"):
                nc.vector.reciprocal(rpb[:], dnb[:])
            nc.gpsimd.tensor_mul(qsl, qsl, rpb[:])

        G = [sb.tile([128, C], BF16, tag="G", bufs=NP, name=f"G{p}")
             for p in range(NP)]

        def emit_g(p):
            for ch in range(2):
                gp = ps.tile([128, 512], F32, tag="kvt", bufs=4,
                             name=f"gp{p}_{ch}")
                nc.tensor.matmul(gp[:], kvbs[p][:],
                                 wo_sb[p][:, ch * 512:(ch + 1) * 512],
                                 start=True, stop=True)
                nc.scalar.copy(G[p][:, ch * 512:(ch + 1) * 512], gp[:])

        # ---- phase 1.5: q projection + ELU, with the G build and the
        # chunk-0 denominator chains interleaved (their inputs are ready
        # once the AllReduce lands mid-phase) ------------------------------
        for p in range(NP):
            if p >= 4:
                emit_g(2 * (p - 4))
                emit_g(2 * (p - 4) + 1)
            if p >= 1:
                emit_scale(p - 1, 0)
            for hh in range(2):
                h0 = hh * 1024
                qp = ps.tile([128, 1024], F32, tag="pp", bufs=2,
                             name=f"qp{p}_{hh}")
                for chk in range(2):
                    for ct in range(KT):
                        nc.tensor.matmul(
                            qp[:, chk * 512:(chk + 1) * 512],
                            wq_sb[ct][:, p * 128:(p + 1) * 128],
                            xs_sb[ct][:, h0 + chk * 512:
                                       h0 + (chk + 1) * 512],
                            start=(ct == 0), stop=(ct == KT - 1))
                qm = tr.tile([128, 1024], BF16, tag="qm", bufs=2,
                             name=f"qm{p}_{hh}")
                qe = tr.tile([128, 1024], BF16, tag="qe", bufs=2,
                             name=f"qe{p}_{hh}")
                HS = (slice(0, 512), slice(512, 1024))
                for hs in HS:
                    nc.vector.tensor_scalar_min(qm[:, hs], qp[:, hs], 0.0)
                for hs in HS:
                    nc.scalar.activation(qe[:, hs], qm[:, hs], Exp)
                for hs in HS:
                    nc.vector.scalar_tensor_tensor(
                        qhat[p][:, h0 + hs.start:h0 + hs.stop], qp[:, hs],
                        0.0, qe[:, hs], Alu.max, Alu.add)

        # ---- phase 2: remaining denominators + output projection -------

        emit_scale(NP - 1, 0)
        for chk in range(S // 512):
            groups = [(mt, ch) for mt in range(chk * 4, chk * 4 + 4)
                      for ch in range(2)]
            for i, (mt, ch) in enumerate(groups):
                if chk + 1 < S // 512 and i < NP:
                    emit_scale(i, chk + 1)
                r0 = mt * 128
                yp = ps.tile([128, 512], F32, tag="pp", bufs=2,
                             name=f"yp{mt}_{ch}")
                for p in range(NP):
                    nc.tensor.matmul(
                        yp[:], qhat[p][:, r0:r0 + 128],
                        G[p][:, ch * 512:(ch + 1) * 512],
                        start=(p == 0), stop=(p == NP - 1))
                ysb = tr.tile([128, 512], BF16, tag="ysb", bufs=3,
                              name=f"ysb{mt}_{ch}")
                nc.scalar.copy(ysb[:], yp[:])
                nc.sync.dma_start(
                    out_d[r0:r0 + 128, ch * 512:(ch + 1) * 512], ysb[:])


def _build(has_bias: bool):
    KT = 9 if has_bias else 8
    KC = KT * 128

    nc = bacc.Bacc("TRN2", target_bir_lowering=False, debug=False,
                   num_devices=N_CORES)
    xt_d = nc.dram_tensor("xt", [KC, S], BF16, kind="ExternalInput").ap()
    wk_d = nc.dram_tensor("wkt", [KC, C], BF16, kind="ExternalInput").ap()
    wv_d = nc.dram_tensor("wvt", [KC, C], BF16, kind="ExternalInput").ap()
    wq_d = nc.dram_tensor("wqt", [KC, C], BF16, kind="ExternalInput").ap()
    wo_d = nc.dram_tensor("wot", [KC, C], BF16, kind="ExternalInput").ap()
    out_d = nc.dram_tensor("out", [S, C], BF16, kind="ExternalOutput").ap()

    with tile.TileContext(nc) as tc:
        _emit(nc, tc, KT, xt_d, wk_d, wv_d, wq_d, wo_d, out_d)
    nc.compile()
    return nc


def _prep_host(inputs, KT):
    """Host-side shard + transpose prep. Returns in_maps for the 8 cores."""
    KC = KT * 128
    npdt = mybir.dt.np(BF16)
    x = np.asarray(inputs["x"], np.float32).reshape(B * T, C)

    def padw(w, b):
        wt = np.ascontiguousarray(np.asarray(w, np.float32).T)  # [Cin, Cout]
        if KC == C:
            return wt.astype(npdt)
        out = np.zeros((KC, C), np.float32)
        out[:C] = wt
        out[C] = np.asarray(b, np.float32)
        return out.astype(npdt)

    wkt = padw(inputs["Wk"], inputs["bk"])
    wvt = padw(inputs["Wv"], inputs["bv"])
    wqt = padw(inputs["Wq"], inputs["bq"])
    wot = padw(inputs["Wo"], np.zeros(C))   # bo applied on host

    in_maps = []
    for c in range(N_CORES):
        sh = x[c * S:(c + 1) * S]
        xt = np.zeros((KC, S), np.float32)
        xt[:C] = sh.T
        if KC > C:
            xt[C] = 1.0
        in_maps.append({
            "xt": np.ascontiguousarray(xt.astype(npdt)),
            "wkt": wkt, "wvt": wvt, "wqt": wqt, "wot": wot,
        })
    return in_maps


def _get_nc(has_bias):
    if has_bias not in _cache:
        _cache[has_bias] = _build(has_bias)
    return _cache[has_bias]


def kernel(**inputs):
    assert np.asarray(inputs["x"]).shape == (B, T, C)
    has_bias = any(
        np.any(np.asarray(inputs[k])) for k in ("bq", "bk", "bv"))
    nc = _get_nc(has_bias)
    in_maps = _prep_host(inputs, 9 if has_bias else 8)
    res = bass_utils.run_bass_kernel_spmd(
        nc, in_maps, core_ids=list(range(N_CORES)))
    y = np.concatenate(
        [np.asarray(res.results[c]["out"], np.float32)
         for c in range(N_CORES)], axis=0)
    y = y.reshape(B, T, C)
    bo = np.asarray(inputs["bo"], np.float32)
    if np.any(bo):
        y = y + bo
    return y


# revision 5
# speedup vs baseline: 1.0565x; 1.0058x over previous
"""Linear attention (ELU+1 feature map) on 8 TRN2 NeuronCores — v10.

Algorithm (see v2): bf16 projections; ELU+1 split DVE/ACT/DVE; per-pair
kvT = vtok^T @ ktok and ksum = ktok^T @ ones2 single-shot into PSUM,
DVE-accumulated in f32 (PSUM accumulation groups interleaved within a bank
are broken on HW); pair AllReduce of kv/ksum overlapped with the q
projection; G_p = kv_p @ Wo_p^T folds kv into the output projection;
KS_p (half-masked free-broadcast ksum) gives the denominator already
broadcast to 128 partitions in ONE matmul; qs = qhat * (1/dnB) in place
(DVE reciprocal + Pool multiply); out = qs^T @ G accumulated over pairs.

Scheduling (v5):
  - ONE PSUM pool for the whole kernel. Opening a new pool makes every new
    tile wait on ALL users of the closed pool (release-boundary barrier,
    measured 5.8us at each phase edge); instead phase 1.5's qp reuses the
    kp/vp tag, G/dnB reuse the kvt tag, yp reuses the pp tag, so cross-
    phase waits are per-slot and land exactly on the pipelined tail.
  - ONE transient SBUF pool for the same reason.
  - kv/ksum matmuls of token-tile tt-1 are emitted after the projections
    of tt; kvt has 4 PSUM bufs so the g2 matmul never waits on the DVE
    aggregation adds.
  - input DMAs ride HWDGE queues (sync for wk, scalar for the rest) in
    consumption order -- gpsimd SWDGE DMAs would occupy the Pool engine.
  - phase-2 unpack runs on Pool during phase 1.5; chunk c+1's
    denominator chains interleave with chunk c's output projection.
"""

import sys
import numpy as np

for _p in ("/opt/trn_rl_repo", "/opt/pypackages"):
    if _p not in sys.path:
        sys.path.append(_p)

import concourse.bacc as bacc
import concourse.mybir as mybir
import concourse.tile as tile
from concourse import bass_utils

F32 = mybir.dt.float32
BF16 = mybir.dt.bfloat16
ACTF = mybir.ActivationFunctionType
Alu = mybir.AluOpType

N_CORES = 8
B, T, C = 4, 4096, 1024
H, D = 16, 64
S = B * T // N_CORES          # 2048 tokens per core
NP = 8                        # head pairs (128 channels each)
TT = S // 128                 # 16 token tiles per core
PSTR = 130                    # kv slot: 128 kvT cols + 2 ksum cols
XCH = 512                     # xs DMA token chunk
OUT_DT = BF16                 # device output dtype (host converts to f32)

_cache = {}


def _emit(nc, tc, KT, xt_d, wk_d, wv_d, wq_d, wo_d, out_d):
    Exp = ACTF.Exp

    with (
        tc.tile_pool(name="wkv", bufs=1) as wkv,
        tc.tile_pool(name="wqo", bufs=1) as wqo,
        tc.tile_pool(name="persist", bufs=1) as sb,
        tc.tile_pool(name="trans", bufs=1) as tr,
        tc.tile_pool(name="psum", bufs=1, space="PSUM") as ps,
        tc.tile_pool(name="dram", bufs=1, space="DRAM") as dram,
    ):
        # ---- input DMAs, in consumption order --------------------------
        # sync: wk (gates the very first matmuls); scalar: everything else
        wk_sb, wv_sb = [], []
        for ct in range(KT):
            w = wkv.tile([128, C], BF16, tag="wkv", bufs=2 * KT,
                         name=f"wk{ct}")
            nc.sync.dma_start(w[:], wk_d[ct * 128:(ct + 1) * 128, :])
            wk_sb.append(w)

        xsall = sb.tile([128, KT * S], BF16, tag="xs", name="xsall")
        xs_sb = [xsall[:, ct * S:(ct + 1) * S] for ct in range(KT)]
        xs3 = xsall.rearrange("p (c s) -> p c s", s=S)
        xt3 = xt_d.rearrange("(c p) s -> p c s", p=128)
        nc.scalar.dma_start(xs3[:, :, 0:XCH], xt3[:, :, 0:XCH])
        for ct in range(KT):
            w = wkv.tile([128, C], BF16, tag="wkv", bufs=2 * KT,
                         name=f"wv{ct}")
            nc.sync.dma_start(w[:], wv_d[ct * 128:(ct + 1) * 128, :])
            wv_sb.append(w)
        nc.sync.dma_start(xs3[:, :, XCH:S], xt3[:, :, XCH:S])
        wqall = wqo.tile([128, KT * C], BF16, tag="wq", name="wqall")
        wq_sb = [wqall[:, ct * C:(ct + 1) * C] for ct in range(KT)]
        nc.sync.dma_start(wqall.rearrange("p (c k) -> p c k", k=C),
                          wq_d.rearrange("(c p) k -> p c k", p=128))
        woall = wqo.tile([128, NP * C], BF16, tag="wo", name="woall")
        wo_sb = [woall[:, p * C:(p + 1) * C] for p in range(NP)]
        nc.sync.dma_start(woall.rearrange("p (c k) -> p c k", k=C),
                          wo_d.rearrange("(c p) k -> p c k",
                                         p=128)[:, 0:NP, :])

        ones2 = sb.tile([128, 2], BF16, tag="ones2", name="ones2")
        nc.gpsimd.memset(ones2[:], 1.0)
        onesks = sb.tile([128, 64], BF16, tag="onesks", name="onesks")
        nc.gpsimd.memset(onesks[:], 1.0)

        kvagg = sb.tile([128, NP * PSTR], F32, tag="kvagg", name="kvagg")
        nc.gpsimd.memset(kvagg[:], 0.0)

        qhat = [sb.tile([128, S], BF16, tag="qhat", bufs=NP, name=f"qhat{p}")
                for p in range(NP)]

        # ---- phase 1: k/v projections + kvT/ksum (kv one tile late) ----
        ktoks = [None] * TT
        vtoks = [None] * TT

        def emit_proj(tt):
            t0 = tt * 128
            xb = [xs_sb[ct][:, t0:t0 + 128] for ct in range(KT)]
            kp = ps.tile([128, C], F32, tag="pp", bufs=2, name=f"kp{tt}")
            for ct in range(KT):       # ct-major: arrival-paced at startup
                for ch in range(2):
                    nc.tensor.matmul(
                        kp[:, ch * 512:(ch + 1) * 512], xb[ct],
                        wk_sb[ct][:, ch * 512:(ch + 1) * 512],
                        start=(ct == 0), stop=(ct == KT - 1))
            vp = ps.tile([128, C], F32, tag="pp", bufs=2, name=f"vp{tt}")
            for ct in range(KT):
                for ch in range(2):
                    nc.tensor.matmul(
                        vp[:, ch * 512:(ch + 1) * 512], xb[ct],
                        wv_sb[ct][:, ch * 512:(ch + 1) * 512],
                        start=(ct == 0), stop=(ct == KT - 1))
            km = tr.tile([128, C], BF16, tag="km", bufs=2, name=f"km{tt}")
            ke = tr.tile([128, C], BF16, tag="ke", bufs=2, name=f"ke{tt}")
            ktok = tr.tile([128, C], BF16, tag="ktok", bufs=3,
                           name=f"ktok{tt}")
            HS = (slice(0, 512), slice(512, 1024))
            for h in HS:
                nc.vector.tensor_scalar_min(km[:, h], kp[:, h], 0.0)
            for h in HS:
                nc.scalar.activation(ke[:, h], km[:, h], Exp)
            for h in HS:
                nc.vector.scalar_tensor_tensor(ktok[:, h], kp[:, h], 0.0,
                                               ke[:, h], Alu.max, Alu.add)
            vtok = tr.tile([128, C], BF16, tag="vtok", bufs=3,
                           name=f"vtok{tt}")
            nc.scalar.copy(vtok[:], vp[:])
            ktoks[tt], vtoks[tt] = ktok, vtok

        def emit_kv(tt):
            ktok, vtok = ktoks[tt], vtoks[tt]
            for g in range(3):
                p0, p1n = 3 * g, min(3 * g + 3, NP)
                kvt = ps.tile([128, (p1n - p0) * PSTR], F32, tag="kvt",
                              bufs=4, name=f"kvt{tt}_{g}",
                              padded_shape=[128, 512])
                for p in range(p0, p1n):
                    j = p - p0
                    nc.tensor.matmul(
                        kvt[:, j * PSTR:j * PSTR + 128],
                        vtok[:, p * 128:(p + 1) * 128],
                        ktok[:, p * 128:(p + 1) * 128],
                        start=True, stop=True)
                    nc.tensor.matmul(
                        kvt[:, j * PSTR + 128:j * PSTR + 130],
                        ktok[:, p * 128:(p + 1) * 128],
                        ones2[:], start=True, stop=True)
                nc.vector.tensor_add(
                    kvagg[:, p0 * PSTR:p1n * PSTR],
                    kvagg[:, p0 * PSTR:p1n * PSTR], kvt[:])

        for tt in range(TT):
            emit_proj(tt)
            if tt > 0:
                emit_kv(tt - 1)
        emit_kv(TT - 1)

        # ---- pair AllReduce (overlaps phase 1.5) -----------------------
        bounce_in = dram.tile([128, NP * PSTR], F32, name="bounce_in")
        bounce_out = dram.tile([128, NP * PSTR], F32, name="bounce_out")
        nc.sync.dma_start(bounce_in[:], kvagg[:])
        nc.gpsimd.collective_compute(
            "AllReduce", Alu.add,
            ins=[bounce_in.opt()], outs=[bounce_out.opt()],
            replica_groups=[[2 * i, 2 * i + 1] for i in range(N_CORES // 2)])
        kvcoll = sb.tile([128, NP * PSTR], F32, tag="kvcoll", name="kvcoll")
        nc.sync.dma_start(kvcoll[:], bounce_out[:])

        # ---- unpack on Pool: runs during phase 1.5 ---------------------
        kvbs, KS = [], []
        for p in range(NP):
            c0 = p * PSTR
            kvb = sb.tile([128, 128], BF16, tag="kvb", bufs=NP,
                          name=f"kvb{p}")
            nc.gpsimd.memset(kvb[:], 0.0)
            nc.gpsimd.tensor_copy(kvb[0:64, 0:64],
                                  kvcoll[0:64, c0:c0 + 64])
            nc.gpsimd.tensor_copy(kvb[64:128, 64:128],
                                  kvcoll[64:128, c0 + 64:c0 + 128])
            kvbs.append(kvb)
            ks = sb.tile([128, 128], BF16, tag="KS", bufs=NP, name=f"KS{p}")
            nc.gpsimd.memset(ks[:], 0.0)
            nc.gpsimd.tensor_scalar_mul(
                ks[0:64, 0:64], onesks[0:64, :],
                kvcoll[0:64, c0 + 128:c0 + 129])
            nc.gpsimd.tensor_scalar_mul(
                ks[64:128, 64:128], onesks[64:128, :],
                kvcoll[64:128, c0 + 128:c0 + 129])
            KS.append(ks)

        # qs = qhat * 1/(KS^T qhat): denominator matmul, reciprocal,
        # in-place Pool multiply. Chunk-0 chains are pre-emitted inside the
        # phase-1.5 tail so the output projection starts immediately.
        def emit_scale(p, chk):
            qsl = qhat[p][:, chk * 512:(chk + 1) * 512]
            dnb = ps.tile([128, 512], F32, tag="kvt", bufs=4,
                          name=f"dnb{p}_{chk}")
            nc.tensor.matmul(dnb[:], KS[p][:], qsl, start=True, stop=True)
            rpb = tr.tile([128, 512], BF16, tag="rpb", bufs=3,
                          name=f"rpb{p}_{chk}")
            with nc.allow_low_precision(reason="recip of denom"):
                nc.vector.reciprocal(rpb[:], dnb[:])
            nc.gpsimd.tensor_mul(qsl, qsl, rpb[:])

        G = [sb.tile([128, C], BF16, tag="G", bufs=NP, name=f"G{p}")
             for p in range(NP)]

        def emit_g(p):
            for ch in range(2):
                gp = ps.tile([128, 512], F32, tag="kvt", bufs=4,
                             name=f"gp{p}_{ch}")
                nc.tensor.matmul(gp[:], kvbs[p][:],
                                 wo_sb[p][:, ch * 512:(ch + 1) * 512],
                                 start=True, stop=True)
                nc.scalar.copy(G[p][:, ch * 512:(ch + 1) * 512], gp[:])

        # ---- phase 1.5: q projection + ELU, with the G build and the
        # chunk-0 denominator chains interleaved (their inputs are ready
        # once the AllReduce lands mid-phase) ------------------------------
        for p in range(NP):
            if p >= 4:
                emit_g(2 * (p - 4))
                emit_g(2 * (p - 4) + 1)
            if p >= 1:
                emit_scale(p - 1, 0)
            for hh in range(2):
                h0 = hh * 1024
                qp = ps.tile([128, 1024], F32, tag="pp", bufs=2,
                             name=f"qp{p}_{hh}")
                for chk in range(2):
                    for ct in range(KT):
                        nc.tensor.matmul(
                            qp[:, chk * 512:(chk + 1) * 512],
                            wq_sb[ct][:, p * 128:(p + 1) * 128],
                            xs_sb[ct][:, h0 + chk * 512:
                                       h0 + (chk + 1) * 512],
                            start=(ct == 0), stop=(ct == KT - 1))
                qm = tr.tile([128, 1024], BF16, tag="qm", bufs=2,
                             name=f"qm{p}_{hh}")
                qe = tr.tile([128, 1024], BF16, tag="qe", bufs=2,
                             name=f"qe{p}_{hh}")
                HS = (slice(0, 512), slice(512, 1024))
                for hs in HS:
                    nc.vector.tensor_scalar_min(qm[:, hs], qp[:, hs], 0.0)
                for hs in HS:
                    nc.scalar.activation(qe[:, hs], qm[:, hs], Exp)
                for hs in HS:
                    nc.vector.scalar_tensor_tensor(
                        qhat[p][:, h0 + hs.start:h0 + hs.stop], qp[:, hs],
                        0.0, qe[:, hs], Alu.max, Alu.add)

        # ---- phase 2: remaining denominators + output projection -------

        emit_scale(NP - 1, 0)
        for chk in range(S // 512):
            groups = [(mt, ch) for mt in range(chk * 4, chk * 4 + 4)
                      for ch in range(2)]
            for i, (mt, ch) in enumerate(groups):
                if chk + 1 < S // 512 and i < NP:
                    emit_scale(i, chk + 1)
                r0 = mt * 128
                yp = ps.tile([128, 512], F32, tag="pp", bufs=2,
                             name=f"yp{mt}_{ch}")
                for p in range(NP):
                    nc.tensor.matmul(
                        yp[:], qhat[p][:, r0:r0 + 128],
                        G[p][:, ch * 512:(ch + 1) * 512],
                        start=(p == 0), stop=(p == NP - 1))
                ysb = tr.tile([128, 512], BF16, tag="ysb", bufs=3,
                              name=f"ysb{mt}_{ch}")
                nc.scalar.copy(ysb[:], yp[:])
                nc.sync.dma_start(
                    out_d[r0:r0 + 128, ch * 512:(ch + 1) * 512], ysb[:])


def _build(has_bias: bool):
    KT = 9 if has_bias else 8
    KC = KT * 128

    nc = bacc.Bacc("TRN2", target_bir_lowering=False, debug=False,
                   num_devices=N_CORES)
    xt_d = nc.dram_tensor("xt", [KC, S], BF16, kind="ExternalInput").ap()
    wk_d = nc.dram_tensor("wkt", [KC, C], BF16, kind="ExternalInput").ap()
    wv_d = nc.dram_tensor("wvt", [KC, C], BF16, kind="ExternalInput").ap()
    wq_d = nc.dram_tensor("wqt", [KC, C], BF16, kind="ExternalInput").ap()
    wo_d = nc.dram_tensor("wot", [KC, C], BF16, kind="ExternalInput").ap()
    out_d = nc.dram_tensor("out", [S, C], BF16, kind="ExternalOutput").ap()

    with tile.TileContext(nc) as tc:
        _emit(nc, tc, KT, xt_d, wk_d, wv_d, wq_d, wo_d, out_d)
    nc.compile()
    return nc


def _prep_host(inputs, KT):
    """Host-side shard + transpose prep. Returns in_maps for the 8 cores."""
    KC = KT * 128
    npdt = mybir.dt.np(BF16)
    x = np.asarray(inputs["x"], np.float32).reshape(B * T, C)

    def padw(w, b):
        wt = np.ascontiguousarray(np.asarray(w, np.float32).T)  # [Cin, Cout]
        if KC == C:
            return wt.astype(npdt)
        out = np.zeros((KC, C), np.float32)
        out[:C] = wt
        out[C] = np.asarray(b, np.float32)
        return out.astype(npdt)

    wkt = padw(inputs["Wk"], inputs["bk"])
    wvt = padw(inputs["Wv"], inputs["bv"])
    wqt = padw(inputs["Wq"], inputs["bq"])
    wot = padw(inputs["Wo"], np.zeros(C))   # bo applied on host

    in_maps = []
    for c in range(N_CORES):
        sh = x[c * S:(c + 1) * S]
        xt = np.zeros((KC, S), np.float32)
        xt[:C] = sh.T
        if KC > C:
            xt[C] = 1.0
        in_maps.append({
            "xt": np.ascontiguousarray(xt.astype(npdt)),
            "wkt": wkt, "wvt": wvt, "wqt": wqt, "wot": wot,
        })
    return in_maps


def _get_nc(has_bias):
    if has_bias not in _cache:
        _cache[has_bias] = _build(has_bias)
    return _cache[has_bias]


def kernel(**inputs):
    assert np.asarray(inputs["x"]).shape == (B, T, C)
    has_bias = any(
        np.any(np.asarray(inputs[k])) for k in ("bq", "bk", "bv"))
    nc = _get_nc(has_bias)
    in_maps = _prep_host(inputs, 9 if has_bias else 8)
    res = bass_utils.run_bass_kernel_spmd(
        nc, in_maps, core_ids=list(range(N_CORES)))
    y = np.concatenate(
        [np.asarray(res.results[c]["out"], np.float32)
         for c in range(N_CORES)], axis=0)
    y = y.reshape(B, T, C)
    bo = np.asarray(inputs["bo"], np.float32)
    if np.any(bo):
        y = y + bo
    return y


# revision 6
# speedup vs baseline: 1.0608x; 1.0041x over previous
"""Linear attention (ELU+1 feature map) on 8 TRN2 NeuronCores — v10.

Algorithm (see v2): bf16 projections; ELU+1 split DVE/ACT/DVE; per-pair
kvT = vtok^T @ ktok and ksum = ktok^T @ ones2 single-shot into PSUM,
DVE-accumulated in f32 (PSUM accumulation groups interleaved within a bank
are broken on HW); pair AllReduce of kv/ksum overlapped with the q
projection; G_p = kv_p @ Wo_p^T folds kv into the output projection;
KS_p (half-masked free-broadcast ksum) gives the denominator already
broadcast to 128 partitions in ONE matmul; qs = qhat * (1/dnB) in place
(DVE reciprocal + Pool multiply); out = qs^T @ G accumulated over pairs.

Scheduling (v5):
  - ONE PSUM pool for the whole kernel. Opening a new pool makes every new
    tile wait on ALL users of the closed pool (release-boundary barrier,
    measured 5.8us at each phase edge); instead phase 1.5's qp reuses the
    kp/vp tag, G/dnB reuse the kvt tag, yp reuses the pp tag, so cross-
    phase waits are per-slot and land exactly on the pipelined tail.
  - ONE transient SBUF pool for the same reason.
  - kv/ksum matmuls of token-tile tt-1 are emitted after the projections
    of tt; kvt has 4 PSUM bufs so the g2 matmul never waits on the DVE
    aggregation adds.
  - input DMAs ride HWDGE queues (sync for wk, scalar for the rest) in
    consumption order -- gpsimd SWDGE DMAs would occupy the Pool engine.
  - phase-2 unpack runs on Pool during phase 1.5; chunk c+1's
    denominator chains interleave with chunk c's output projection.
"""

import sys
import numpy as np

for _p in ("/opt/trn_rl_repo", "/opt/pypackages"):
    if _p not in sys.path:
        sys.path.append(_p)

import concourse.bacc as bacc
import concourse.mybir as mybir
import concourse.tile as tile
from concourse import bass_utils

F32 = mybir.dt.float32
BF16 = mybir.dt.bfloat16
ACTF = mybir.ActivationFunctionType
Alu = mybir.AluOpType

N_CORES = 8
B, T, C = 4, 4096, 1024
H, D = 16, 64
S = B * T // N_CORES          # 2048 tokens per core
NP = 8                        # head pairs (128 channels each)
TT = S // 128                 # 16 token tiles per core
PSTR = 130                    # kv slot: 128 kvT cols + 2 ksum cols
XCH = 512                     # xs DMA token chunk
OUT_DT = BF16                 # device output dtype (host converts to f32)

_cache = {}


def _emit(nc, tc, KT, xt_d, wk_d, wv_d, wq_d, wo_d, out_d):
    Exp = ACTF.Exp

    with (
        tc.tile_pool(name="wkv", bufs=1) as wkv,
        tc.tile_pool(name="wqo", bufs=1) as wqo,
        tc.tile_pool(name="persist", bufs=1) as sb,
        tc.tile_pool(name="trans", bufs=1) as tr,
        tc.tile_pool(name="psum", bufs=1, space="PSUM") as ps,
        tc.tile_pool(name="dram", bufs=1, space="DRAM") as dram,
    ):
        # ---- input DMAs, in consumption order --------------------------
        # sync: wk (gates the very first matmuls); scalar: everything else
        wk_sb, wv_sb = [], []
        for ct in range(KT):
            w = wkv.tile([128, C], BF16, tag="wkv", bufs=2 * KT,
                         name=f"wk{ct}")
            nc.sync.dma_start(w[:], wk_d[ct * 128:(ct + 1) * 128, :])
            wk_sb.append(w)

        xsall = sb.tile([128, KT * S], BF16, tag="xs", name="xsall")
        xs_sb = [xsall[:, ct * S:(ct + 1) * S] for ct in range(KT)]
        xs3 = xsall.rearrange("p (c s) -> p c s", s=S)
        xt3 = xt_d.rearrange("(c p) s -> p c s", p=128)
        nc.scalar.dma_start(xs3[:, :, 0:256], xt3[:, :, 0:256])
        nc.scalar.dma_start(xs3[:, :, 256:XCH], xt3[:, :, 256:XCH])
        for ct in range(KT):
            w = wkv.tile([128, C], BF16, tag="wkv", bufs=2 * KT,
                         name=f"wv{ct}")
            nc.sync.dma_start(w[:], wv_d[ct * 128:(ct + 1) * 128, :])
            wv_sb.append(w)
        nc.sync.dma_start(xs3[:, :, XCH:S], xt3[:, :, XCH:S])
        wqall = wqo.tile([128, KT * C], BF16, tag="wq", name="wqall")
        wq_sb = [wqall[:, ct * C:(ct + 1) * C] for ct in range(KT)]
        nc.sync.dma_start(wqall.rearrange("p (c k) -> p c k", k=C),
                          wq_d.rearrange("(c p) k -> p c k", p=128))
        woall = wqo.tile([128, NP * C], BF16, tag="wo", name="woall")
        wo_sb = [woall[:, p * C:(p + 1) * C] for p in range(NP)]
        nc.sync.dma_start(woall.rearrange("p (c k) -> p c k", k=C),
                          wo_d.rearrange("(c p) k -> p c k",
                                         p=128)[:, 0:NP, :])

        ones2 = sb.tile([128, 2], BF16, tag="ones2", name="ones2")
        nc.gpsimd.memset(ones2[:], 1.0)
        onesks = sb.tile([128, 64], BF16, tag="onesks", name="onesks")
        nc.gpsimd.memset(onesks[:], 1.0)

        kvagg = sb.tile([128, NP * PSTR], F32, tag="kvagg", name="kvagg")
        nc.gpsimd.memset(kvagg[:], 0.0)

        qhat = [sb.tile([128, S], BF16, tag="qhat", bufs=NP, name=f"qhat{p}")
                for p in range(NP)]

        # ---- phase 1: k/v projections + kvT/ksum (kv one tile late) ----
        ktoks = [None] * TT
        vtoks = [None] * TT

        def emit_proj(tt):
            t0 = tt * 128
            xb = [xs_sb[ct][:, t0:t0 + 128] for ct in range(KT)]
            kp = ps.tile([128, C], F32, tag="pp", bufs=2, name=f"kp{tt}")
            for ct in range(KT):       # ct-major: arrival-paced at startup
                for ch in range(2):
                    nc.tensor.matmul(
                        kp[:, ch * 512:(ch + 1) * 512], xb[ct],
                        wk_sb[ct][:, ch * 512:(ch + 1) * 512],
                        start=(ct == 0), stop=(ct == KT - 1))
            vp = ps.tile([128, C], F32, tag="pp", bufs=2, name=f"vp{tt}")
            for ct in range(KT):
                for ch in range(2):
                    nc.tensor.matmul(
                        vp[:, ch * 512:(ch + 1) * 512], xb[ct],
                        wv_sb[ct][:, ch * 512:(ch + 1) * 512],
                        start=(ct == 0), stop=(ct == KT - 1))
            km = tr.tile([128, C], BF16, tag="km", bufs=2, name=f"km{tt}")
            ke = tr.tile([128, C], BF16, tag="ke", bufs=2, name=f"ke{tt}")
            ktok = tr.tile([128, C], BF16, tag="ktok", bufs=3,
                           name=f"ktok{tt}")
            HS = (slice(0, 512), slice(512, 1024))
            for h in HS:
                nc.vector.tensor_scalar_min(km[:, h], kp[:, h], 0.0)
            for h in HS:
                nc.scalar.activation(ke[:, h], km[:, h], Exp)
            for h in HS:
                nc.vector.scalar_tensor_tensor(ktok[:, h], kp[:, h], 0.0,
                                               ke[:, h], Alu.max, Alu.add)
            vtok = tr.tile([128, C], BF16, tag="vtok", bufs=3,
                           name=f"vtok{tt}")
            nc.scalar.copy(vtok[:], vp[:])
            ktoks[tt], vtoks[tt] = ktok, vtok

        def emit_kv(tt):
            ktok, vtok = ktoks[tt], vtoks[tt]
            for g in range(3):
                p0, p1n = 3 * g, min(3 * g + 3, NP)
                kvt = ps.tile([128, (p1n - p0) * PSTR], F32, tag="kvt",
                              bufs=4, name=f"kvt{tt}_{g}",
                              padded_shape=[128, 512])
                for p in range(p0, p1n):
                    j = p - p0
                    nc.tensor.matmul(
                        kvt[:, j * PSTR:j * PSTR + 128],
                        vtok[:, p * 128:(p + 1) * 128],
                        ktok[:, p * 128:(p + 1) * 128],
                        start=True, stop=True)
                    nc.tensor.matmul(
                        kvt[:, j * PSTR + 128:j * PSTR + 130],
                        ktok[:, p * 128:(p + 1) * 128],
                        ones2[:], start=True, stop=True)
                nc.vector.tensor_add(
                    kvagg[:, p0 * PSTR:p1n * PSTR],
                    kvagg[:, p0 * PSTR:p1n * PSTR], kvt[:])

        for tt in range(TT):
            emit_proj(tt)
            if tt > 0:
                emit_kv(tt - 1)
        emit_kv(TT - 1)

        # ---- pair AllReduce (overlaps phase 1.5) -----------------------
        bounce_in = dram.tile([128, NP * PSTR], F32, name="bounce_in")
        bounce_out = dram.tile([128, NP * PSTR], F32, name="bounce_out")
        nc.sync.dma_start(bounce_in[:], kvagg[:])
        nc.gpsimd.collective_compute(
            "AllReduce", Alu.add,
            ins=[bounce_in.opt()], outs=[bounce_out.opt()],
            replica_groups=[[2 * i, 2 * i + 1] for i in range(N_CORES // 2)])
        kvcoll = sb.tile([128, NP * PSTR], F32, tag="kvcoll", name="kvcoll")
        nc.sync.dma_start(kvcoll[:], bounce_out[:])

        # ---- unpack on Pool: runs during phase 1.5 ---------------------
        kvbs, KS = [], []
        for p in range(NP):
            c0 = p * PSTR
            kvb = sb.tile([128, 128], BF16, tag="kvb", bufs=NP,
                          name=f"kvb{p}")
            nc.gpsimd.memset(kvb[:], 0.0)
            nc.gpsimd.tensor_copy(kvb[0:64, 0:64],
                                  kvcoll[0:64, c0:c0 + 64])
            nc.gpsimd.tensor_copy(kvb[64:128, 64:128],
                                  kvcoll[64:128, c0 + 64:c0 + 128])
            kvbs.append(kvb)
            ks = sb.tile([128, 128], BF16, tag="KS", bufs=NP, name=f"KS{p}")
            nc.gpsimd.memset(ks[:], 0.0)
            nc.gpsimd.tensor_scalar_mul(
                ks[0:64, 0:64], onesks[0:64, :],
                kvcoll[0:64, c0 + 128:c0 + 129])
            nc.gpsimd.tensor_scalar_mul(
                ks[64:128, 64:128], onesks[64:128, :],
                kvcoll[64:128, c0 + 128:c0 + 129])
            KS.append(ks)

        # qs = qhat * 1/(KS^T qhat): denominator matmul, reciprocal,
        # in-place Pool multiply. Chunk-0 chains are pre-emitted inside the
        # phase-1.5 tail so the output projection starts immediately.
        def emit_scale(p, chk):
            qsl = qhat[p][:, chk * 512:(chk + 1) * 512]
            dnb = ps.tile([128, 512], F32, tag="kvt", bufs=4,
                          name=f"dnb{p}_{chk}")
            nc.tensor.matmul(dnb[:], KS[p][:], qsl, start=True, stop=True)
            rpb = tr.tile([128, 512], BF16, tag="rpb", bufs=3,
                          name=f"rpb{p}_{chk}")
            with nc.allow_low_precision(reason="recip of denom"):
                nc.vector.reciprocal(rpb[:], dnb[:])
            nc.gpsimd.tensor_mul(qsl, qsl, rpb[:])

        G = [sb.tile([128, C], BF16, tag="G", bufs=NP, name=f"G{p}")
             for p in range(NP)]

        def emit_g(p):
            for ch in range(2):
                gp = ps.tile([128, 512], F32, tag="kvt", bufs=4,
                             name=f"gp{p}_{ch}")
                nc.tensor.matmul(gp[:], kvbs[p][:],
                                 wo_sb[p][:, ch * 512:(ch + 1) * 512],
                                 start=True, stop=True)
                nc.scalar.copy(G[p][:, ch * 512:(ch + 1) * 512], gp[:])

        # ---- phase 1.5: q projection + ELU, with the G build and the
        # chunk-0 denominator chains interleaved (their inputs are ready
        # once the AllReduce lands mid-phase) ------------------------------
        for p in range(NP):
            if p >= 4:
                emit_g(2 * (p - 4))
                emit_g(2 * (p - 4) + 1)
            if p >= 2:
                emit_scale(p - 2, 0)
            for hh in range(2):
                if p == NP - 1 and hh == 1:
                    emit_scale(NP - 2, 0)
                h0 = hh * 1024
                qp = ps.tile([128, 1024], F32, tag="pp", bufs=2,
                             name=f"qp{p}_{hh}")
                for chk in range(2):
                    for ct in range(KT):
                        nc.tensor.matmul(
                            qp[:, chk * 512:(chk + 1) * 512],
                            wq_sb[ct][:, p * 128:(p + 1) * 128],
                            xs_sb[ct][:, h0 + chk * 512:
                                       h0 + (chk + 1) * 512],
                            start=(ct == 0), stop=(ct == KT - 1))
                qm = tr.tile([128, 1024], BF16, tag="qm", bufs=2,
                             name=f"qm{p}_{hh}")
                qe = tr.tile([128, 1024], BF16, tag="qe", bufs=2,
                             name=f"qe{p}_{hh}")
                HS = (slice(0, 512), slice(512, 1024))
                for hs in HS:
                    nc.vector.tensor_scalar_min(qm[:, hs], qp[:, hs], 0.0)
                for hs in HS:
                    nc.scalar.activation(qe[:, hs], qm[:, hs], Exp)
                for hs in HS:
                    nc.vector.scalar_tensor_tensor(
                        qhat[p][:, h0 + hs.start:h0 + hs.stop], qp[:, hs],
                        0.0, qe[:, hs], Alu.max, Alu.add)

        # ---- phase 2: remaining denominators + output projection -------

        emit_scale(NP - 1, 0)
        for chk in range(S // 512):
            groups = [(mt, ch) for mt in range(chk * 4, chk * 4 + 4)
                      for ch in range(2)]
            for i, (mt, ch) in enumerate(groups):
                if chk + 1 < S // 512 and i < NP:
                    emit_scale(i, chk + 1)
                r0 = mt * 128
                yp = ps.tile([128, 512], F32, tag="pp", bufs=2,
                             name=f"yp{mt}_{ch}")
                for p in range(NP):
                    nc.tensor.matmul(
                        yp[:], qhat[p][:, r0:r0 + 128],
                        G[p][:, ch * 512:(ch + 1) * 512],
                        start=(p == 0), stop=(p == NP - 1))
                ysb = tr.tile([128, 512], BF16, tag="ysb", bufs=3,
                              name=f"ysb{mt}_{ch}")
                nc.scalar.copy(ysb[:], yp[:])
                nc.sync.dma_start(
                    out_d[r0:r0 + 128, ch * 512:(ch + 1) * 512], ysb[:])


def _build(has_bias: bool):
    KT = 9 if has_bias else 8
    KC = KT * 128

    nc = bacc.Bacc("TRN2", target_bir_lowering=False, debug=False,
                   num_devices=N_CORES)
    xt_d = nc.dram_tensor("xt", [KC, S], BF16, kind="ExternalInput").ap()
    wk_d = nc.dram_tensor("wkt", [KC, C], BF16, kind="ExternalInput").ap()
    wv_d = nc.dram_tensor("wvt", [KC, C], BF16, kind="ExternalInput").ap()
    wq_d = nc.dram_tensor("wqt", [KC, C], BF16, kind="ExternalInput").ap()
    wo_d = nc.dram_tensor("wot", [KC, C], BF16, kind="ExternalInput").ap()
    out_d = nc.dram_tensor("out", [S, C], BF16, kind="ExternalOutput").ap()

    with tile.TileContext(nc) as tc:
        _emit(nc, tc, KT, xt_d, wk_d, wv_d, wq_d, wo_d, out_d)
    nc.compile()
    return nc


def _prep_host(inputs, KT):
    """Host-side shard + transpose prep. Returns in_maps for the 8 cores."""
    KC = KT * 128
    npdt = mybir.dt.np(BF16)
    x = np.asarray(inputs["x"], np.float32).reshape(B * T, C)

    def padw(w, b):
        wt = np.ascontiguousarray(np.asarray(w, np.float32).T)  # [Cin, Cout]
        if KC == C:
            return wt.astype(npdt)
        out = np.zeros((KC, C), np.float32)
        out[:C] = wt
        out[C] = np.asarray(b, np.float32)
        return out.astype(npdt)

    wkt = padw(inputs["Wk"], inputs["bk"])
    wvt = padw(inputs["Wv"], inputs["bv"])
    wqt = padw(inputs["Wq"], inputs["bq"])
    wot = padw(inputs["Wo"], np.zeros(C))   # bo applied on host

    in_maps = []
    for c in range(N_CORES):
        sh = x[c * S:(c + 1) * S]
        xt = np.zeros((KC, S), np.float32)
        xt[:C] = sh.T
        if KC > C:
            xt[C] = 1.0
        in_maps.append({
            "xt": np.ascontiguousarray(xt.astype(npdt)),
            "wkt": wkt, "wvt": wvt, "wqt": wqt, "wot": wot,
        })
    return in_maps


def _get_nc(has_bias):
    if has_bias not in _cache:
        _cache[has_bias] = _build(has_bias)
    return _cache[has_bias]


def kernel(**inputs):
    assert np.asarray(inputs["x"]).shape == (B, T, C)
    has_bias = any(
        np.any(np.asarray(inputs[k])) for k in ("bq", "bk", "bv"))
    nc = _get_nc(has_bias)
    in_maps = _prep_host(inputs, 9 if has_bias else 8)
    res = bass_utils.run_bass_kernel_spmd(
        nc, in_maps, core_ids=list(range(N_CORES)))
    y = np.concatenate(
        [np.asarray(res.results[c]["out"], np.float32)
         for c in range(N_CORES)], axis=0)
    y = y.reshape(B, T, C)
    bo = np.asarray(inputs["bo"], np.float32)
    if np.any(bo):
        y = y + bo
    return y


# revision 7
# speedup vs baseline: 1.0649x; 1.0038x over previous
"""Linear attention (ELU+1 feature map) on 8 TRN2 NeuronCores — v10.

Algorithm (see v2): bf16 projections; ELU+1 split DVE/ACT/DVE; per-pair
kvT = vtok^T @ ktok and ksum = ktok^T @ ones2 single-shot into PSUM,
DVE-accumulated in f32 (PSUM accumulation groups interleaved within a bank
are broken on HW); pair AllReduce of kv/ksum overlapped with the q
projection; G_p = kv_p @ Wo_p^T folds kv into the output projection;
KS_p (half-masked free-broadcast ksum) gives the denominator already
broadcast to 128 partitions in ONE matmul; qs = qhat * (1/dnB) in place
(DVE reciprocal + Pool multiply); out = qs^T @ G accumulated over pairs.

Scheduling (v5):
  - ONE PSUM pool for the whole kernel. Opening a new pool makes every new
    tile wait on ALL users of the closed pool (release-boundary barrier,
    measured 5.8us at each phase edge); instead phase 1.5's qp reuses the
    kp/vp tag, G/dnB reuse the kvt tag, yp reuses the pp tag, so cross-
    phase waits are per-slot and land exactly on the pipelined tail.
  - ONE transient SBUF pool for the same reason.
  - kv/ksum matmuls of token-tile tt-1 are emitted after the projections
    of tt; kvt has 4 PSUM bufs so the g2 matmul never waits on the DVE
    aggregation adds.
  - input DMAs ride HWDGE queues (sync for wk, scalar for the rest) in
    consumption order -- gpsimd SWDGE DMAs would occupy the Pool engine.
  - phase-2 unpack runs on Pool during phase 1.5; chunk c+1's
    denominator chains interleave with chunk c's output projection.
"""

import sys
import numpy as np

for _p in ("/opt/trn_rl_repo", "/opt/pypackages"):
    if _p not in sys.path:
        sys.path.append(_p)

import concourse.bacc as bacc
import concourse.mybir as mybir
import concourse.tile as tile
from concourse import bass_utils

F32 = mybir.dt.float32
BF16 = mybir.dt.bfloat16
ACTF = mybir.ActivationFunctionType
Alu = mybir.AluOpType

N_CORES = 8
B, T, C = 4, 4096, 1024
H, D = 16, 64
S = B * T // N_CORES          # 2048 tokens per core
NP = 8                        # head pairs (128 channels each)
TT = S // 128                 # 16 token tiles per core
PSTR = 130                    # kv slot: 128 kvT cols + 2 ksum cols
XCH = 512                     # xs DMA token chunk
OUT_DT = BF16                 # device output dtype (host converts to f32)

_cache = {}


def _emit(nc, tc, KT, xt_d, wk_d, wv_d, wq_d, wo_d, out_d):
    Exp = ACTF.Exp

    with (
        tc.tile_pool(name="wkv", bufs=1) as wkv,
        tc.tile_pool(name="wqo", bufs=1) as wqo,
        tc.tile_pool(name="persist", bufs=1) as sb,
        tc.tile_pool(name="trans", bufs=1) as tr,
        tc.tile_pool(name="psum", bufs=1, space="PSUM") as ps,
        tc.tile_pool(name="dram", bufs=1, space="DRAM") as dram,
    ):
        # ---- input DMAs, in consumption order --------------------------
        # sync: wk (gates the very first matmuls); scalar: everything else
        wk_sb, wv_sb = [], []
        for ct in range(KT):
            w = wkv.tile([128, C], BF16, tag="wkv", bufs=2 * KT,
                         name=f"wk{ct}")
            nc.sync.dma_start(w[:], wk_d[ct * 128:(ct + 1) * 128, :])
            wk_sb.append(w)

        xsall = sb.tile([128, KT * S], BF16, tag="xs", name="xsall")
        xs_sb = [xsall[:, ct * S:(ct + 1) * S] for ct in range(KT)]
        xs3 = xsall.rearrange("p (c s) -> p c s", s=S)
        xt3 = xt_d.rearrange("(c p) s -> p c s", p=128)
        nc.scalar.dma_start(xs3[:, :, 0:256], xt3[:, :, 0:256])
        nc.scalar.dma_start(xs3[:, :, 256:XCH], xt3[:, :, 256:XCH])
        for ct in range(KT):
            w = wkv.tile([128, C], BF16, tag="wkv", bufs=2 * KT,
                         name=f"wv{ct}")
            nc.sync.dma_start(w[:], wv_d[ct * 128:(ct + 1) * 128, :])
            wv_sb.append(w)
        nc.sync.dma_start(xs3[:, :, XCH:S], xt3[:, :, XCH:S])
        wqall = wqo.tile([128, KT * C], BF16, tag="wq", name="wqall")
        wq_sb = [wqall[:, ct * C:(ct + 1) * C] for ct in range(KT)]
        nc.sync.dma_start(wqall.rearrange("p (c k) -> p c k", k=C),
                          wq_d.rearrange("(c p) k -> p c k", p=128))
        woall = wqo.tile([128, NP * C], BF16, tag="wo", name="woall")
        wo_sb = [woall[:, p * C:(p + 1) * C] for p in range(NP)]
        nc.sync.dma_start(woall.rearrange("p (c k) -> p c k", k=C),
                          wo_d.rearrange("(c p) k -> p c k",
                                         p=128)[:, 0:NP, :])

        ones2 = sb.tile([128, 2], BF16, tag="ones2", name="ones2")
        nc.gpsimd.memset(ones2[:], 1.0)
        onesks = sb.tile([128, 64], BF16, tag="onesks", name="onesks")
        nc.gpsimd.memset(onesks[:], 1.0)

        kvagg = sb.tile([128, NP * PSTR], F32, tag="kvagg", name="kvagg")
        nc.gpsimd.memset(kvagg[:], 0.0)

        qhat = [sb.tile([128, S], BF16, tag="qhat", bufs=NP, name=f"qhat{p}")
                for p in range(NP)]

        # ---- phase 1: k/v projections + kvT/ksum (kv one tile late) ----
        ktoks = [None] * TT
        vtoks = [None] * TT

        def emit_proj(tt):
            t0 = tt * 128
            xb = [xs_sb[ct][:, t0:t0 + 128] for ct in range(KT)]
            kp = ps.tile([128, C], F32, tag="pp", bufs=2, name=f"kp{tt}")
            for ct in range(KT):       # ct-major: arrival-paced at startup
                for ch in range(2):
                    nc.tensor.matmul(
                        kp[:, ch * 512:(ch + 1) * 512], xb[ct],
                        wk_sb[ct][:, ch * 512:(ch + 1) * 512],
                        start=(ct == 0), stop=(ct == KT - 1))
            vp = ps.tile([128, C], F32, tag="pp", bufs=2, name=f"vp{tt}")
            for ct in range(KT):
                for ch in range(2):
                    nc.tensor.matmul(
                        vp[:, ch * 512:(ch + 1) * 512], xb[ct],
                        wv_sb[ct][:, ch * 512:(ch + 1) * 512],
                        start=(ct == 0), stop=(ct == KT - 1))
            km = tr.tile([128, C], BF16, tag="km", bufs=2, name=f"km{tt}")
            ke = tr.tile([128, C], BF16, tag="ke", bufs=2, name=f"ke{tt}")
            ktok = tr.tile([128, C], BF16, tag="ktok", bufs=3,
                           name=f"ktok{tt}")
            HS = (slice(0, 512), slice(512, 1024))
            for h in HS:
                nc.vector.tensor_scalar_min(km[:, h], kp[:, h], 0.0)
            for h in HS:
                nc.scalar.activation(ke[:, h], km[:, h], Exp)
            for h in HS:
                nc.vector.scalar_tensor_tensor(ktok[:, h], kp[:, h], 0.0,
                                               ke[:, h], Alu.max, Alu.add)
            vtok = tr.tile([128, C], BF16, tag="vtok", bufs=3,
                           name=f"vtok{tt}")
            nc.scalar.copy(vtok[:], vp[:])
            ktoks[tt], vtoks[tt] = ktok, vtok

        def emit_kv(tt):
            ktok, vtok = ktoks[tt], vtoks[tt]
            for g in range(3):
                p0, p1n = 3 * g, min(3 * g + 3, NP)
                kvt = ps.tile([128, (p1n - p0) * PSTR], F32, tag="kvt",
                              bufs=4, name=f"kvt{tt}_{g}",
                              padded_shape=[128, 512])
                for p in range(p0, p1n):
                    j = p - p0
                    nc.tensor.matmul(
                        kvt[:, j * PSTR:j * PSTR + 128],
                        vtok[:, p * 128:(p + 1) * 128],
                        ktok[:, p * 128:(p + 1) * 128],
                        start=True, stop=True)
                    nc.tensor.matmul(
                        kvt[:, j * PSTR + 128:j * PSTR + 130],
                        ktok[:, p * 128:(p + 1) * 128],
                        ones2[:], start=True, stop=True)
                nc.vector.tensor_add(
                    kvagg[:, p0 * PSTR:p1n * PSTR],
                    kvagg[:, p0 * PSTR:p1n * PSTR], kvt[:])

        for tt in range(TT):
            emit_proj(tt)
            if tt > 0:
                emit_kv(tt - 1)
        emit_kv(TT - 1)

        # ---- pair AllReduce (overlaps phase 1.5) -----------------------
        bounce_in = dram.tile([128, NP * PSTR], F32, name="bounce_in")
        bounce_out = dram.tile([128, NP * PSTR], F32, name="bounce_out")
        nc.sync.dma_start(bounce_in[:], kvagg[:])
        nc.gpsimd.collective_compute(
            "AllReduce", Alu.add,
            ins=[bounce_in.opt()], outs=[bounce_out.opt()],
            replica_groups=[[2 * i, 2 * i + 1] for i in range(N_CORES // 2)])
        kvcoll = sb.tile([128, NP * PSTR], F32, tag="kvcoll", name="kvcoll")
        nc.sync.dma_start(kvcoll[:], bounce_out[:])

        # ---- unpack on Pool: runs during phase 1.5 ---------------------
        kvbs, KS = [], []
        for p in range(NP):
            c0 = p * PSTR
            kvb = sb.tile([128, 128], BF16, tag="kvb", bufs=NP,
                          name=f"kvb{p}")
            nc.gpsimd.memset(kvb[:], 0.0)
            nc.gpsimd.tensor_copy(kvb[0:64, 0:64],
                                  kvcoll[0:64, c0:c0 + 64])
            nc.gpsimd.tensor_copy(kvb[64:128, 64:128],
                                  kvcoll[64:128, c0 + 64:c0 + 128])
            kvbs.append(kvb)
            ks = sb.tile([128, 128], BF16, tag="KS", bufs=NP, name=f"KS{p}")
            nc.gpsimd.memset(ks[:], 0.0)
            nc.gpsimd.tensor_scalar_mul(
                ks[0:64, 0:64], onesks[0:64, :],
                kvcoll[0:64, c0 + 128:c0 + 129])
            nc.gpsimd.tensor_scalar_mul(
                ks[64:128, 64:128], onesks[64:128, :],
                kvcoll[64:128, c0 + 128:c0 + 129])
            KS.append(ks)

        # qs = qhat * 1/(KS^T qhat): denominator matmul, reciprocal,
        # in-place Pool multiply. Chunk-0 chains are pre-emitted inside the
        # phase-1.5 tail so the output projection starts immediately.
        def emit_scale(p, chk, mul_eng=None):
            qsl = qhat[p][:, chk * 512:(chk + 1) * 512]
            dnb = ps.tile([128, 512], F32, tag="kvt", bufs=4,
                          name=f"dnb{p}_{chk}")
            nc.tensor.matmul(dnb[:], KS[p][:], qsl, start=True, stop=True)
            rpb = tr.tile([128, 512], BF16, tag="rpb", bufs=3,
                          name=f"rpb{p}_{chk}")
            with nc.allow_low_precision(reason="recip of denom"):
                nc.vector.reciprocal(rpb[:], dnb[:])
            # Pool's 0.42-efficiency multiply is 1.1us; the last chunk-0
            # chains gate the first output group, so they ride DVE (0.66us)
            (mul_eng or nc.gpsimd).tensor_mul(qsl, qsl, rpb[:])

        G = [sb.tile([128, C], BF16, tag="G", bufs=NP, name=f"G{p}")
             for p in range(NP)]

        def emit_g(p):
            for ch in range(2):
                gp = ps.tile([128, 512], F32, tag="kvt", bufs=4,
                             name=f"gp{p}_{ch}")
                nc.tensor.matmul(gp[:], kvbs[p][:],
                                 wo_sb[p][:, ch * 512:(ch + 1) * 512],
                                 start=True, stop=True)
                nc.scalar.copy(G[p][:, ch * 512:(ch + 1) * 512], gp[:])

        # ---- phase 1.5: q projection + ELU, with the G build and the
        # chunk-0 denominator chains interleaved (their inputs are ready
        # once the AllReduce lands mid-phase) ------------------------------
        for p in range(NP):
            if p >= 4:
                emit_g(2 * (p - 4))
                emit_g(2 * (p - 4) + 1)
            if p >= 2:
                emit_scale(p - 2, 0)
            for hh in range(2):
                if p == NP - 1 and hh == 1:
                    emit_scale(NP - 2, 0, mul_eng=nc.vector)
                h0 = hh * 1024
                qp = ps.tile([128, 1024], F32, tag="pp", bufs=2,
                             name=f"qp{p}_{hh}")
                for chk in range(2):
                    for ct in range(KT):
                        nc.tensor.matmul(
                            qp[:, chk * 512:(chk + 1) * 512],
                            wq_sb[ct][:, p * 128:(p + 1) * 128],
                            xs_sb[ct][:, h0 + chk * 512:
                                       h0 + (chk + 1) * 512],
                            start=(ct == 0), stop=(ct == KT - 1))
                qm = tr.tile([128, 1024], BF16, tag="qm", bufs=2,
                             name=f"qm{p}_{hh}")
                qe = tr.tile([128, 1024], BF16, tag="qe", bufs=2,
                             name=f"qe{p}_{hh}")
                HS = (slice(0, 512), slice(512, 1024))
                for hs in HS:
                    nc.vector.tensor_scalar_min(qm[:, hs], qp[:, hs], 0.0)
                for hs in HS:
                    nc.scalar.activation(qe[:, hs], qm[:, hs], Exp)
                for hs in HS:
                    nc.vector.scalar_tensor_tensor(
                        qhat[p][:, h0 + hs.start:h0 + hs.stop], qp[:, hs],
                        0.0, qe[:, hs], Alu.max, Alu.add)

        # ---- phase 2: remaining denominators + output projection -------

        emit_scale(NP - 1, 0, mul_eng=nc.vector)
        for chk in range(S // 512):
            groups = [(mt, ch) for mt in range(chk * 4, chk * 4 + 4)
                      for ch in range(2)]
            for i, (mt, ch) in enumerate(groups):
                if chk + 1 < S // 512 and i < NP:
                    emit_scale(i, chk + 1)
                r0 = mt * 128
                yp = ps.tile([128, 512], F32, tag="kvt", bufs=4,
                             name=f"yp{mt}_{ch}")
                for p in range(NP):
                    nc.tensor.matmul(
                        yp[:], qhat[p][:, r0:r0 + 128],
                        G[p][:, ch * 512:(ch + 1) * 512],
                        start=(p == 0), stop=(p == NP - 1))
                ysb = tr.tile([128, 512], BF16, tag="ysb", bufs=3,
                              name=f"ysb{mt}_{ch}")
                nc.scalar.copy(ysb[:], yp[:])
                nc.sync.dma_start(
                    out_d[r0:r0 + 128, ch * 512:(ch + 1) * 512], ysb[:])


def _build(has_bias: bool):
    KT = 9 if has_bias else 8
    KC = KT * 128

    nc = bacc.Bacc("TRN2", target_bir_lowering=False, debug=False,
                   num_devices=N_CORES)
    xt_d = nc.dram_tensor("xt", [KC, S], BF16, kind="ExternalInput").ap()
    wk_d = nc.dram_tensor("wkt", [KC, C], BF16, kind="ExternalInput").ap()
    wv_d = nc.dram_tensor("wvt", [KC, C], BF16, kind="ExternalInput").ap()
    wq_d = nc.dram_tensor("wqt", [KC, C], BF16, kind="ExternalInput").ap()
    wo_d = nc.dram_tensor("wot", [KC, C], BF16, kind="ExternalInput").ap()
    out_d = nc.dram_tensor("out", [S, C], BF16, kind="ExternalOutput").ap()

    with tile.TileContext(nc) as tc:
        _emit(nc, tc, KT, xt_d, wk_d, wv_d, wq_d, wo_d, out_d)
    nc.compile()
    return nc


def _prep_host(inputs, KT):
    """Host-side shard + transpose prep. Returns in_maps for the 8 cores."""
    KC = KT * 128
    npdt = mybir.dt.np(BF16)
    x = np.asarray(inputs["x"], np.float32).reshape(B * T, C)

    def padw(w, b):
        wt = np.ascontiguousarray(np.asarray(w, np.float32).T)  # [Cin, Cout]
        if KC == C:
            return wt.astype(npdt)
        out = np.zeros((KC, C), np.float32)
        out[:C] = wt
        out[C] = np.asarray(b, np.float32)
        return out.astype(npdt)

    wkt = padw(inputs["Wk"], inputs["bk"])
    wvt = padw(inputs["Wv"], inputs["bv"])
    wqt = padw(inputs["Wq"], inputs["bq"])
    wot = padw(inputs["Wo"], np.zeros(C))   # bo applied on host

    in_maps = []
    for c in range(N_CORES):
        sh = x[c * S:(c + 1) * S]
        xt = np.zeros((KC, S), np.float32)
        xt[:C] = sh.T
        if KC > C:
            xt[C] = 1.0
        in_maps.append({
            "xt": np.ascontiguousarray(xt.astype(npdt)),
            "wkt": wkt, "wvt": wvt, "wqt": wqt, "wot": wot,
        })
    return in_maps


def _get_nc(has_bias):
    if has_bias not in _cache:
        _cache[has_bias] = _build(has_bias)
    return _cache[has_bias]


def kernel(**inputs):
    assert np.asarray(inputs["x"]).shape == (B, T, C)
    has_bias = any(
        np.any(np.asarray(inputs[k])) for k in ("bq", "bk", "bv"))
    nc = _get_nc(has_bias)
    in_maps = _prep_host(inputs, 9 if has_bias else 8)
    res = bass_utils.run_bass_kernel_spmd(
        nc, in_maps, core_ids=list(range(N_CORES)))
    y = np.concatenate(
        [np.asarray(res.results[c]["out"], np.float32)
         for c in range(N_CORES)], axis=0)
    y = y.reshape(B, T, C)
    bo = np.asarray(inputs["bo"], np.float32)
    if np.any(bo):
        y = y + bo
    return y
